# revision 50
# baseline (speedup 1.0000x reference)
"""Multi-head attention (B=4, L=2048, D=1024, H=16) on 8 trn2 NeuronCores.

Sharding: core c = 2*b + g handles batch b and head-group g (8 heads = 512 dims).
Each core computes Q/K/V projections for its group, attention for its 8 heads,
and a partial output projection ctx_g @ Wo[g*512:(g+1)*512, :].  The host sums
the two group partials per batch.

Causal fast path (bf16 PE inputs; ~224.1us tile-cost-model span per core):
  QT, KT  : (512, 2048) feature-major bf16 (4 tiles of (128, L), 2 heads/tile)
  V       : 16 token tiles (128, 8*65) bf16; col 64 of each head = ones column
            that accumulates the softmax denominator Z during the ctx matmul
  scores  : per (head-pair, key-tile) a paired PSUM tile (128, 2, 512) holding
            both heads' score blocks; diagonal key-tiles are column-trimmed to
            the causally-live range and get a (128,2,128) triangular mask add
            on DVE; ONE exp per pair on ACT (bias = padd mask per key)
  ctx     : token-major accumulation — out (128 tok, 65) per 128-query chunk
            (moving dim = 65 features, so ctx matmul cost is ~0.5x of the
            feature-major form); Z lands as a per-partition column, so the
            softmax division is a plain DVE tensor_scalar (no broadcast);
            a tiny PE transpose (identity matmul) restores feature-major
            ctxt for the output projection
  out     : ctxt.T @ Wo chunks into PSUM, DVE/ACT copy, DMA to DRAM f32;
            outproj(t) emission is deferred two blocks and proj(t+1) pieces
            are interleaved between head-pair blocks, so both slot into the
            ACT-bound exp stretches as greedy PE filler; for the final
            block a 2-pass split (jt0-2 into an SBUF partial early, jt3 +
            identity-matmul fold-in late) shortens the tail
  softmax : no max-subtraction (scores are O(3); masked entries hit exp(-1e9)=0)
  x DMAs ride the gpsimd queue, weights/outputs the sync queue, so trigger
  latencies overlap; startup interleaves weight and x chunks per use order.

Non-causal fallback: generic-mask fp32r variant (mask supplied as data).
"""

import sys

if "/opt/trn_rl_repo" not in sys.path:
    sys.path.insert(0, "/opt/trn_rl_repo")

import numpy as np

B, L, D, H = 4, 2048, 1024, 16
G = 2                # head-groups == cores per batch
DG = D // G          # 512 dims per group
HG = H // G          # 8 heads per group
DH = D // H          # 64
NCORES = B * G
NT = L // 512        # query 512-blocks
NKT = L // 128       # key 128-tiles
ND = D // 128        # contraction chunks over input dim
NJ = DG // 128       # dcol tiles per group (2 heads each)

MM_DTYPE = "bfloat16"

_cache = {}


def _build_causal():
    import concourse.bass as bass
    import concourse.tile as tile
    from concourse import bacc, mybir

    f32 = mybir.dt.float32
    f32r = mybir.dt.float32r
    bf16 = mybir.dt.bfloat16
    AF = mybir.ActivationFunctionType

    nc = bacc.Bacc("TRN2")

    xq_d = nc.dram_tensor("xqt", [D, L], bf16, kind="ExternalInput")
    xk_d = nc.dram_tensor("xkt", [D, L], bf16, kind="ExternalInput")
    xv_d = nc.dram_tensor("xvt", [D, L], bf16, kind="ExternalInput")
    wq_d = nc.dram_tensor("wq", [D, DG], bf16, kind="ExternalInput")
    wk_d = nc.dram_tensor("wk", [D, DG], bf16, kind="ExternalInput")
    wv_d = nc.dram_tensor("wv", [D, DG], bf16, kind="ExternalInput")
    bq_d = nc.dram_tensor("bq", [128, NJ], f32, kind="ExternalInput")
    bk_d = nc.dram_tensor("bk", [128, NJ], f32, kind="ExternalInput")
    bv_d = nc.dram_tensor("bv", [DG], f32, kind="ExternalInput")
    wo_d = nc.dram_tensor("wo", [DG, D], bf16, kind="ExternalInput")
    dtri_d = nc.dram_tensor("dtri", [128, 128], f32, kind="ExternalInput")
    pdk_d = nc.dram_tensor("paddk", [128, NKT], f32, kind="ExternalInput")
    id_d = nc.dram_tensor("ident", [128, 128], bf16, kind="ExternalInput")
    out_d = nc.dram_tensor("out", [L, D], f32, kind="ExternalOutput")

    with tile.TileContext(nc) as tc, (
        __import__("contextlib").ExitStack()) as ctx:
        ep = ctx.enter_context
        wpool = ep(tc.tile_pool(name="persist", bufs=1))
        qkpool = ep(tc.tile_pool(name="qk", bufs=1))
        vpool = ep(tc.tile_pool(name="vaug", bufs=1))
        xpool = ep(tc.tile_pool(name="xin", bufs=1))
        scp = ep(tc.tile_pool(name="scps", bufs=2, space="PSUM"))
        cpp = ep(tc.tile_pool(name="ctxps", bufs=2, space="PSUM"))
        gpp = ep(tc.tile_pool(name="gps", bufs=2, space="PSUM"))
        pepool = ep(tc.tile_pool(name="pexp", bufs=3))
        ctpool = ep(tc.tile_pool(name="ctxt", bufs=1))
        rzpool = ep(tc.tile_pool(name="rz", bufs=2))
        opool = ep(tc.tile_pool(name="outsb", bufs=3))

        # ---- persistent weights/biases; DMA order == transfer order, so
        # issue exactly what the first projections need first.
        wq_bg = wpool.tile([128, ND, DG], bf16, tag="wqb", name="wq_bg")
        wk_bg = wpool.tile([128, ND, DG], bf16, tag="wkb", name="wk_bg")
        wv_bg = wpool.tile([128, ND, DG], bf16, tag="wvb", name="wv_bg")
        bq_sb = wpool.tile([128, NJ], f32, tag="bq")
        bk_sb = wpool.tile([128, NJ], f32, tag="bk")
        bv_sb = wpool.tile([128, DG], f32, tag="bv")
        xt = {}
        for name in ("q", "k", "v"):
            xt[name] = [
                xpool.tile([128, ND, 512], bf16, tag=f"x{name}", name=f"x{name}",
                           bufs=3)
                for _ in range(NT)]

        def dma_w_half(wd, wb, hx):
            nc.sync.dma_start(
                out=wb[:, 4 * hx:4 * (hx + 1), :],
                in_=wd[512 * hx:512 * (hx + 1), :].rearrange(
                    "(i p) n -> p i n", p=128))

        def dma_x_chunk(name, xd, t, i, n):
            ts = slice(512 * t, 512 * (t + 1))
            nc.gpsimd.dma_start(
                out=xt[name][t][:, i:i + n, :],
                in_=xd[128 * i:128 * (i + n), ts].rearrange(
                    "(i p) n -> p i n", p=128))

        def dma_x(name, xd, t):
            nc.gpsimd.dma_start(
                out=xt[name][t],
                in_=xd[:, 512 * t:512 * (t + 1)].rearrange(
                    "(i p) n -> p i n", p=128))

        # startup: interleave weight halves with x chunks so the first
        # projection matmuls start as early as possible
        def dma_w_q(wd, wb, i):
            nc.sync.dma_start(
                out=wb[:, i:i + 2, :],
                in_=wd[128 * i:128 * (i + 2), :].rearrange(
                    "(i p) n -> p i n", p=128))

        def dma_w_1(wd, wb, i):
            nc.sync.dma_start(
                out=wb[:, i:i + 1, :],
                in_=wd[128 * i:128 * (i + 1), :].rearrange(
                    "(i p) n -> p i n", p=128))

        dma_w_1(wq_d, wq_bg, 0)
        dma_x_chunk("q", xq_d, 0, 0, 1)
        dma_w_1(wq_d, wq_bg, 1)
        dma_x_chunk("q", xq_d, 0, 1, 1)
        dma_w_q(wq_d, wq_bg, 2)
        dma_x_chunk("q", xq_d, 0, 2, 2)
        dma_w_half(wq_d, wq_bg, 1)
        dma_x_chunk("q", xq_d, 0, 4, 2)
        dma_x_chunk("q", xq_d, 0, 6, 2)
        nc.sync.dma_start(out=bq_sb, in_=bq_d[:, :])
        dma_w_half(wk_d, wk_bg, 0)
        dma_x_chunk("k", xk_d, 0, 0, 2)
        dma_x_chunk("k", xk_d, 0, 2, 2)
        dma_w_half(wk_d, wk_bg, 1)
        dma_x_chunk("k", xk_d, 0, 4, 2)
        dma_x_chunk("k", xk_d, 0, 6, 2)
        nc.sync.dma_start(out=bk_sb, in_=bk_d[:, :])

        dtri = wpool.tile([128, 128], f32, tag="dtri")
        nc.sync.dma_start(out=dtri, in_=dtri_d[:, :])
        pdk_sb = wpool.tile([128, NKT], f32, tag="pdk")
        nc.sync.dma_start(out=pdk_sb, in_=pdk_d[:, :])

        dma_w_half(wv_d, wv_bg, 0)
        dma_x_chunk("v", xv_d, 0, 0, 4)
        dma_w_half(wv_d, wv_bg, 1)
        bv_ap = bv_d[:]
        bv_bcast = bass.AP(
            tensor=bv_ap.tensor, offset=bv_ap.offset,
            ap=[[0, 128]] + list(bv_ap.ap))
        nc.sync.dma_start(out=bv_sb, in_=bv_bcast)
        dma_x_chunk("v", xv_d, 0, 4, 4)

        ident = wpool.tile([128, 128], bf16, tag="ident")
        nc.sync.dma_start(out=ident, in_=id_d[:, :])

        wq_sb = [wq_bg[:, i, :] for i in range(ND)]
        wk_sb = [wk_bg[:, i, :] for i in range(ND)]
        wv_sb = [wv_bg[:, i, :] for i in range(ND)]

        # triangular mask broadcast over the head-pair dim: [128, 2, 128]
        dt_ap = dtri[:, :]
        dtri2 = bass.AP(
            tensor=dt_ap.tensor, offset=dt_ap.offset,
            ap=[dt_ap.ap[0], [0, 2], dt_ap.ap[1]])

        qt_sb = [qkpool.tile([128, L], bf16, tag=f"qt{j}", name="qt_sb")
                 for j in range(NJ)]
        kt_sb = [qkpool.tile([128, L], bf16, tag=f"kt{j}", name="kt_sb")
                 for j in range(NJ)]
        vaug = [vpool.tile([128, HG * 65], bf16, tag=f"va{k}", name="vaug")
                for k in range(NKT)]
        for kt in range(NKT):
            v3 = vaug[kt].rearrange("p (h d) -> p h d", h=HG)
            nc.vector.memset(v3[:, :, 64:65], 1.0)


        wo_bg = wpool.tile([128, NJ, D], bf16, tag="wob", name="wo_bg")
        wo_sb = [wo_bg[:, j, :] for j in range(NJ)]

        def emit_proj(t, only=None):
            ts = slice(512 * t, 512 * (t + 1))
            for name, w_sb, b_sb, dest in (
                ("q", wq_sb, bq_sb, qt_sb),
                ("k", wk_sb, bk_sb, kt_sb),
                ("v", wv_sb, None, None),
            ):
                if only is not None and name != only:
                    continue
                xts = xt[name][t]
                if dest is not None:  # Q/K: feature-major output
                    for j in range(NJ):
                        ps = gpp.tile([128, 512], f32, tag="gp")
                        for i in range(ND):
                            nc.tensor.matmul(
                                out=ps,
                                lhsT=w_sb[i][:, 128 * j:128 * (j + 1)],
                                rhs=xts[:, i, :],
                                start=(i == 0), stop=(i == ND - 1))
                        nc.vector.tensor_scalar_add(
                            out=dest[j][:, ts], in0=ps,
                            scalar1=b_sb[:, j:j + 1])
                else:  # V: token-major output, bv add fused in copy-out
                    for s in range(4):
                        ps = gpp.tile([128, 512], f32, tag="gp")
                        for i in range(ND):
                            nc.tensor.matmul(
                                out=ps,
                                lhsT=xts[:, i, 128 * s:128 * (s + 1)],
                                rhs=wv_sb[i],
                                start=(i == 0), stop=(i == ND - 1))
                        kt = 4 * t + s
                        v3 = vaug[kt].rearrange("p (h d) -> p h d", h=HG)
                        nc.vector.tensor_add(
                            v3[:, :, 0:64],
                            ps.rearrange("p (h d) -> p h d", h=HG),
                            bv_sb.rearrange("p (h d) -> p h d", h=HG))

        def emit_attn(t, fillers=()):
            qs0 = 512 * t
            nkt_t = 4 * t + 4
            ctxt = [ctpool.tile([128, 512], bf16, tag=f"ct{j}", name="ctxt",
                                bufs=4) for j in range(NJ)]
            for hp in range(NJ):
                if hp < len(fillers) and fillers[hp] is not None:
                    fillers[hp]()
                # token-major ctx accumulators: bank X holds query chunks
                # (2X, 2X+1); cols h*65+64 accumulate the softmax denom Z
                # (padded to a full 2048B bank so matmuls stay bank-contained)
                cu = [cpp.tile([128, 2, 256], f32, tag="cu", name="cu",
                               bufs=2) for _ in range(2)]
                for kt in range(nkt_t):
                    ks = slice(128 * kt, 128 * (kt + 1))
                    j = kt - 4 * t  # >= 0 on diagonal tiles
                    o = 128 * j if j >= 0 else 0
                    pair = scp.tile([128, 2, 512], f32, tag="sc", name="scores")
                    for half in range(2):
                        ro = 64 * half
                        nc.tensor.matmul(
                            out=pair[:, half, o:512],
                            lhsT=kt_sb[hp][ro:ro + 64, ks],
                            rhs=qt_sb[hp][ro:ro + 64, qs0 + o:qs0 + 512],
                            start=True, stop=True)
                    if j >= 0:
                        nc.vector.tensor_add(
                            pair[:, :, o:o + 128], pair[:, :, o:o + 128], dtri2)
                    pe = pepool.tile([128, 2, 512], bf16, tag="pe", name="pexp",
                                     bufs=6)
                    nc.scalar.activation(
                        out=pe[:, :, o:512], in_=pair[:, :, o:512],
                        func=AF.Exp, bias=pdk_sb[:, kt:kt + 1])
                    c0 = max(j, 0)
                    for c in range(c0, 4):
                        for half in range(2):
                            h = 2 * hp + half
                            # start zeroes the full 2KB PSUM zero-region, so
                            # only the first matmul into each bank sets it
                            nc.tensor.matmul(
                                out=cu[c // 2][:, c % 2, 65 * half:65 * half + 65],
                                lhsT=pe[:, half, 128 * c:128 * (c + 1)],
                                rhs=vaug[kt][:, 65 * h:65 * (h + 1)],
                                start=(kt == 0 and half == 0 and c % 2 == 0),
                                stop=(kt == 4 * t + 2 * (c // 2) + 1
                                      and c % 2 == 1 and half == 1))
                # normalize: Z sits as per-partition columns; recip + scalar
                # multiply, then PE-transpose back to feature-major ctxt
                cub = ctpool.tile([128, 4, 128], bf16, tag="cub", bufs=2)
                last = (t == NT - 1 and hp == NJ - 1)
                for X in range(2):
                    cu4 = cu[X][:, :, 0:130].rearrange(
                        "p c (h f) -> p c h f", f=65)
                    rz = rzpool.tile([128, 2, 2], f32, tag="rz", bufs=4)
                    nc.vector.reciprocal(out=rz, in_=cu4[:, :, :, 64])
                    for cc in range(2):
                        for half in range(2):
                            co = cub[:, 2 * X + cc, 64 * half:64 * half + 64]
                            if last and half == 1:
                                # ACT is idle at the very end; split the
                                # normalize between both engines
                                nc.scalar.mul(
                                    co, cu4[:, cc, half, 0:64],
                                    rz[:, cc, half:half + 1])
                            else:
                                nc.vector.tensor_scalar_mul(
                                    out=co, in0=cu4[:, cc, half, 0:64],
                                    scalar1=rz[:, cc, half:half + 1])
                tr = cpp.tile([128, 1024], bf16, tag="cu", name="tr")
                for c in range(4):
                    nc.tensor.matmul(
                        out=tr[:, 128 * c:128 * (c + 1)],
                        lhsT=cub[:, c, :],
                        rhs=ident,
                        is_transpose=True,
                        start=(c == 0), stop=(c == 3), skip_group_check=True)
                nc.vector.tensor_copy(out=ctxt[hp], in_=tr[:, 0:512])
            return ctxt

        def emit_outproj(t, ctxt, split=False):
            # split=True: accumulate jt0-2 into an SBUF partial early (fills
            # PE while the last head-pairs' exps drain), only jt3 + add +
            # DMA remain after the final ctxt lands
            parts = {}
            if split:
                for s in range(4):
                    for e in range(2):
                        es = slice(512 * e, 512 * (e + 1))
                        ps = gpp.tile([128, 512], f32, tag="gp")
                        for jt in range(3):
                            nc.tensor.matmul(
                                out=ps,
                                lhsT=ctxt[jt][:, 128 * s:128 * (s + 1)],
                                rhs=wo_sb[jt][:, es],
                                start=(jt == 0), stop=(jt == 2))
                        oa = opool.tile([128, 512], bf16, tag="oa", bufs=8)
                        nc.vector.tensor_copy(out=oa, in_=ps)
                        parts[s, e] = oa
            if split:
                # final pass on sc pair tiles (scores are done): jt3 plus the
                # SBUF partial folded back via identity matmuls, all on PE
                for s in range(4):
                    pr = scp.tile([128, 2, 512], f32, tag="sc", name="opr")
                    for e in range(2):
                        es = slice(512 * e, 512 * (e + 1))
                        nc.tensor.matmul(
                            out=pr[:, e, :],
                            lhsT=ctxt[3][:, 128 * s:128 * (s + 1)],
                            rhs=wo_sb[3][:, es],
                            start=True, stop=False)
                        nc.tensor.matmul(
                            out=pr[:, e, :], lhsT=ident, rhs=parts[s, e],
                            start=False, stop=True)
                    r0 = 512 * t + 128 * s
                    for e in range(2):
                        es = slice(512 * e, 512 * (e + 1))
                        ob = opool.tile([128, 512], f32, tag="ob", bufs=6)
                        if e == 0:
                            nc.vector.tensor_copy(out=ob, in_=pr[:, e, :])
                        else:
                            nc.scalar.copy(out=ob, in_=pr[:, e, :])
                        q = nc.gpsimd if e == 1 else nc.sync
                        q.dma_start(out=out_d[r0:r0 + 128, es], in_=ob)
            else:
                for s in range(4):
                    for e in range(2):
                        es = slice(512 * e, 512 * (e + 1))
                        ps = gpp.tile([128, 512], f32, tag="gp")
                        for jt in range(NJ):
                            nc.tensor.matmul(
                                out=ps,
                                lhsT=ctxt[jt][:, 128 * s:128 * (s + 1)],
                                rhs=wo_sb[jt][:, es],
                                start=(jt == 0), stop=(jt == NJ - 1))
                        ob = opool.tile([128, 512], f32, tag="ob", bufs=6)
                        if e == 0:
                            nc.vector.tensor_copy(out=ob, in_=ps)
                        else:
                            nc.scalar.copy(out=ob, in_=ps)
                        r0 = 512 * t + 128 * s
                        nc.sync.dma_start(out=out_d[r0:r0 + 128, es], in_=ob)

        emit_proj(0)
        for name, xd in (("q", xq_d), ("k", xk_d), ("v", xv_d)):
            dma_x(name, xd, 1)
        nc.sync.dma_start(
            out=wo_bg, in_=wo_d[:, :].rearrange("(j p) n -> p j n", p=128))
        # outproj(t) is deferred two attention blocks: it becomes greedy PE
        # filler for the ACT-bound stretches of the last attention blocks
        pend = []
        for t in range(NT):
            # proj(t+1) pieces are interleaved between head-pair blocks so
            # their priority slots them into the ACT-bound exp stretches
            if t + 1 < NT:
                fillers = [None] + [
                    (lambda n=n: emit_proj(t + 1, only=n))
                    for n in ("q", "k", "v")]
            else:
                fillers = ()
            pend.append((t, emit_attn(t, fillers)))
            if t + 1 < NT and t + 2 < NT:
                for name, xd in (("q", xq_d), ("k", xk_d), ("v", xv_d)):
                    dma_x(name, xd, t + 2)
            if t >= 2:
                emit_outproj(*pend.pop(0))
        while len(pend) > 1:
            emit_outproj(*pend.pop(0))
        emit_outproj(*pend.pop(0), split=True)

    nc.finalize()
    return nc


def _build_generic():
    """Non-causal fallback: generic additive mask as data, fp32r PE."""
    import concourse.bass as bass
    import concourse.tile as tile
    from concourse import bacc, mybir

    f32 = mybir.dt.float32
    AF = mybir.ActivationFunctionType
    mdt = mybir.dt.float32r

    nc = bacc.Bacc("TRN2")

    xqt = nc.dram_tensor("xqt", [D, L], mdt, kind="ExternalInput")
    xkt = nc.dram_tensor("xkt", [D, L], mdt, kind="ExternalInput")
    xvt = nc.dram_tensor("xvt", [D, L], mdt, kind="ExternalInput")
    wq_d = nc.dram_tensor("wq", [D, DG], mdt, kind="ExternalInput")
    wk_d = nc.dram_tensor("wk", [D, DG], mdt, kind="ExternalInput")
    wv_d = nc.dram_tensor("wv", [D, DG], mdt, kind="ExternalInput")
    bq_d = nc.dram_tensor("bq", [128, NJ], f32, kind="ExternalInput")
    bk_d = nc.dram_tensor("bk", [128, NJ], f32, kind="ExternalInput")
    bv_d = nc.dram_tensor("bv", [DG], f32, kind="ExternalInput")
    wo_d = nc.dram_tensor("wo", [DG, D], mdt, kind="ExternalInput")
    msk_d = nc.dram_tensor("maskt", [L, L], f32, kind="ExternalInput")
    out_d = nc.dram_tensor("out", [L, D], f32, kind="ExternalOutput")

    with tile.TileContext(nc) as tc, (
        __import__("contextlib").ExitStack()) as ctx:
        ep = ctx.enter_context
        wpool = ep(tc.tile_pool(name="persist", bufs=1))
        qkpool = ep(tc.tile_pool(name="qk", bufs=1))
        vpool = ep(tc.tile_pool(name="vaug", bufs=1))
        zdpool = ep(tc.tile_pool(name="zdram", bufs=4, space="DRAM"))
        mmp = ep(tc.tile_pool(name="mm", bufs=4, space="PSUM"))
        ctxp = ep(tc.tile_pool(name="ctxps", bufs=2, space="PSUM"))
        wop = ep(tc.tile_pool(name="wops", bufs=2, space="PSUM"))
        ppool = ep(tc.tile_pool(name="pexp", bufs=3))
        ctpool = ep(tc.tile_pool(name="ctxt", bufs=4))
        cupool = ep(tc.tile_pool(name="ctxu", bufs=2))
        rbpool = ep(tc.tile_pool(name="rbc", bufs=1))
        opool = ep(tc.tile_pool(name="outsb", bufs=2))

        wo_bg = wpool.tile([128, NJ, D], mdt, tag="wob", name="wo_bg")
        nc.sync.dma_start(
            out=wo_bg, in_=wo_d[:, :].rearrange("(j p) n -> p j n", p=128))
        wo_sb = [wo_bg[:, j, :] for j in range(NJ)]
        qt_sb = [qkpool.tile([128, L], mdt, tag=f"qt{j}", name="qt_sb") for j in range(NJ)]
        kt_sb = [qkpool.tile([128, L], mdt, tag=f"kt{j}", name="kt_sb") for j in range(NJ)]
        vaug = [vpool.tile([128, HG * 65], mdt, tag=f"va{k}", name="vaug") for k in range(NKT)]
        ones8 = wpool.tile([128, HG, 1], f32, tag="ones8")
        nc.vector.memset(ones8, 1.0)
        for kt in range(NKT):
            v3 = vaug[kt].rearrange("p (h d) -> p h d", h=HG)
            nc.scalar.copy(out=v3[:, :, 64:65], in_=ones8)

        wq_sb = wk_sb = wv_sb = bq_sb = bk_sb = bv_sb = None

        def setup_w3(w3pool):
            nonlocal wq_sb, wk_sb, wv_sb, bq_sb, bk_sb, bv_sb
            wq_bg = w3pool.tile([128, ND, DG], mdt, tag="wqb", name="wq_bg")
            wk_bg = w3pool.tile([128, ND, DG], mdt, tag="wkb", name="wk_bg")
            wv_bg = w3pool.tile([128, ND, DG], mdt, tag="wvb", name="wv_bg")
            for wd, wb in ((wq_d, wq_bg), (wk_d, wk_bg), (wv_d, wv_bg)):
                nc.sync.dma_start(
                    out=wb, in_=wd[:, :].rearrange("(i p) n -> p i n", p=128))
            wq_sb = [wq_bg[:, i, :] for i in range(ND)]
            wk_sb = [wk_bg[:, i, :] for i in range(ND)]
            wv_sb = [wv_bg[:, i, :] for i in range(ND)]
            bq_sb = w3pool.tile([128, NJ], f32, tag="bq")
            bk_sb = w3pool.tile([128, NJ], f32, tag="bk")
            nc.sync.dma_start(out=bq_sb, in_=bq_d[:, :])
            nc.sync.dma_start(out=bk_sb, in_=bk_d[:, :])
            bv_sb = w3pool.tile([128, DG], f32, tag="bv")
            bv_ap = bv_d[:]
            bv_bcast = bass.AP(
                tensor=bv_ap.tensor, offset=bv_ap.offset,
                ap=[[0, 128]] + list(bv_ap.ap))
            nc.sync.dma_start(out=bv_sb, in_=bv_bcast)

        def emit_proj(t, xpool):
            ts = slice(512 * t, 512 * (t + 1))
            for xd, w_sb, b_sb, dest in (
                (xqt, wq_sb, bq_sb, qt_sb),
                (xkt, wk_sb, bk_sb, kt_sb),
                (xvt, wv_sb, None, None),
            ):
                xts = []
                for hx in range(4):
                    xt_bg = xpool.tile([128, ND // 4, 512], mdt, tag="xt",
                                       name="xt_bg", bufs=3)
                    rs = slice(256 * hx, 256 * (hx + 1))
                    nc.sync.dma_start(
                        out=xt_bg,
                        in_=xd[rs, ts].rearrange("(i p) n -> p i n", p=128))
                    xts.extend(xt_bg[:, i, :] for i in range(ND // 4))
                if dest is not None:  # Q/K: feature-major output
                    for j in range(NJ):
                        ps = mmp.tile([128, 512], f32, tag="mm")
                        for i in range(ND):
                            nc.tensor.matmul(
                                out=ps,
                                lhsT=w_sb[i][:, 128 * j:128 * (j + 1)],
                                rhs=xts[i],
                                start=(i == 0), stop=(i == ND - 1))
                        nc.scalar.activation(
                            out=dest[j][:, ts], in_=ps, func=AF.Identity,
                            bias=b_sb[:, j:j + 1])
                else:  # V: token-major output, bv add fused in copy-out
                    for s in range(4):
                        ps = mmp.tile([128, 512], f32, tag="mm")
                        for i in range(ND):
                            nc.tensor.matmul(
                                out=ps,
                                lhsT=xts[i][:, 128 * s:128 * (s + 1)],
                                rhs=wv_sb[i],
                                start=(i == 0), stop=(i == ND - 1))
                        kt = 4 * t + s
                        v3 = vaug[kt].rearrange("p (h d) -> p h d", h=HG)
                        nc.vector.tensor_add(
                            v3[:, :, 0:64],
                            ps.rearrange("p (h d) -> p h d", h=HG),
                            bv_sb.rearrange("p (h d) -> p h d", h=HG))

        def emit_attn(t, mpool):
            qs = slice(512 * t, 512 * (t + 1))
            msk = []
            for hkt in range(4):
                msk_bg = mpool.tile([128, NKT // 4, 512], f32, tag="msk",
                                    name="msk_bg", bufs=6)
                rs = slice(512 * hkt, 512 * (hkt + 1))
                nc.sync.dma_start(
                    out=msk_bg,
                    in_=msk_d[rs, qs].rearrange("(k p) n -> p k n", p=128))
                msk.extend(msk_bg[:, kt, :] for kt in range(NKT // 4))
            ctxt = [ctpool.tile([128, 512], mdt, tag="ct", name="ctxt") for _ in range(NJ)]
            for hp in range(NJ):
                jt = hp
                ctx_ab = [ctxp.tile([65, 512], f32, tag="ctx", name="ctx_ab") for _ in range(2)]
                for kt in range(NKT):
                    ks = slice(128 * kt, 128 * (kt + 1))
                    pexp = []
                    for half in range(2):
                        ro = 64 * half
                        ps = mmp.tile([128, 512], f32, tag="mm")
                        nc.tensor.matmul(
                            out=ps,
                            lhsT=kt_sb[jt][ro:ro + 64, ks],
                            rhs=qt_sb[jt][ro:ro + 64, qs],
                            start=True, stop=True)
                        nc.vector.tensor_add(ps, ps, msk[kt])
                        pe = ppool.tile([128, 512], mdt, tag="pexp")
                        nc.scalar.activation(out=pe, in_=ps, func=AF.Exp, bias=0.0)
                        pexp.append(pe)
                    for half in range(2):
                        h = 2 * hp + half
                        nc.tensor.matmul(
                            out=ctx_ab[half],
                            lhsT=vaug[kt][:, 65 * h:65 * (h + 1)],
                            rhs=pexp[half],
                            start=(kt == 0), stop=(kt == NKT - 1))
                for half in range(2):
                    ro = 64 * half
                    cu = cupool.tile([65, 512], f32, tag="cu")
                    nc.vector.tensor_copy(out=cu, in_=ctx_ab[half])
                    nc.vector.reciprocal(out=cu[64:65, :], in_=cu[64:65, :])
                    zd = zdpool.tile([1, 512], f32, tag="zd", name="zd")
                    nc.sync.dma_start(out=zd, in_=cu[64:65, :])
                    zrow = zd[0, :]
                    rb_src = bass.AP(
                        tensor=zrow.tensor, offset=zrow.offset,
                        ap=[[0, 64]] + list(zrow.ap))
                    rb = rbpool.tile([64, 512], f32, tag="rb")
                    nc.sync.dma_start(out=rb, in_=rb_src)
                    nc.vector.tensor_mul(
                        ctxt[jt][ro:ro + 64, :], cu[0:64, :], rb)
            for s in range(4):
                for e in range(2):
                    es = slice(512 * e, 512 * (e + 1))
                    ps = wop.tile([128, 512], f32, tag="wo")
                    for jt in range(NJ):
                        nc.tensor.matmul(
                            out=ps,
                            lhsT=ctxt[jt][:, 128 * s:128 * (s + 1)],
                            rhs=wo_sb[jt][:, es],
                            start=(jt == 0), stop=(jt == NJ - 1))
                    ob = opool.tile([128, 512], f32, tag="ob")
                    nc.vector.tensor_copy(out=ob, in_=ps)
                    r0 = 512 * t + 128 * s
                    nc.sync.dma_start(out=out_d[r0:r0 + 128, es], in_=ob)

        with (
            tc.tile_pool(name="w3", bufs=1) as w3pool,
            tc.tile_pool(name="xin", bufs=1) as xpool,
        ):
            setup_w3(w3pool)
            for t in range(NT):
                emit_proj(t, xpool)
        mpool = ep(tc.tile_pool(name="msk", bufs=1))
        for t in range(NT):
            emit_attn(t, mpool)

    nc.finalize()
    return nc


def _get_nc(causal):
    if causal not in _cache:
        _cache[causal] = _build_causal() if causal else _build_generic()
    return _cache[causal]


last_result = None


def _is_causal(attn_mask):
    tri = np.tril(np.ones((L, L), bool))
    expect = np.where(tri, np.float32(0.0), np.float32(-1e9))
    return np.array_equal(attn_mask, expect)


def _causal_in_maps(inp):
    from ml_dtypes import bfloat16

    scale = 1.0 / np.sqrt(np.float32(DH))
    wq_s = (inp["Wq"].astype(np.float32) * scale).astype(bfloat16)
    bq_s = (inp["bq"].astype(np.float32) * scale)
    wk_s = inp["Wk"].astype(np.float32).astype(bfloat16)
    wv_s = inp["Wv"].astype(np.float32).astype(bfloat16)
    wo_s = inp["Wo"].astype(np.float32).astype(bfloat16)
    padd = inp["padd_mask"].astype(np.float32)

    kk = np.arange(128)[:, None]
    qq = np.arange(128)[None, :]
    dtri = np.where(qq >= kk, np.float32(0.0),
                    np.float32(-1e9)).astype(np.float32)
    ident = np.eye(128, dtype=np.float32).astype(bfloat16)

    in_maps = []
    for b in range(B):
        xq = np.ascontiguousarray(
            inp["encodings_for_q"][b].astype(np.float32).T).astype(bfloat16)
        xk = np.ascontiguousarray(
            inp["encodings_for_k"][b].astype(np.float32).T).astype(bfloat16)
        xv = np.ascontiguousarray(
            inp["encodings_for_v"][b].astype(np.float32).T).astype(bfloat16)
        pdk = np.ascontiguousarray(padd[b].reshape(NKT, 128).T)
        for g in range(G):
            gs = slice(DG * g, DG * (g + 1))
            in_maps.append({
                "xqt": xq, "xkt": xk, "xvt": xv,
                "wq": np.ascontiguousarray(wq_s[:, gs]),
                "wk": np.ascontiguousarray(wk_s[:, gs]),
                "wv": np.ascontiguousarray(wv_s[:, gs]),
                "bq": np.ascontiguousarray(bq_s[gs].reshape(NJ, 128).T),
                "bk": np.ascontiguousarray(
                    inp["bk"].astype(np.float32)[gs].reshape(NJ, 128).T),
                "bv": np.ascontiguousarray(inp["bv"].astype(np.float32)[gs]),
                "wo": np.ascontiguousarray(wo_s[gs, :]),
                "dtri": dtri,
                "paddk": pdk,
                "ident": ident,
            })
    return in_maps


def _generic_in_maps(inp):
    scale = 1.0 / np.sqrt(np.float32(DH))
    wq_s = (inp["Wq"] * scale).astype(np.float32)
    bq_s = (inp["bq"] * scale).astype(np.float32)
    padd = inp["padd_mask"].astype(np.float32)
    maskT = np.ascontiguousarray(inp["attn_mask"].astype(np.float32).T)

    in_maps = []
    for b in range(B):
        xq = np.ascontiguousarray(inp["encodings_for_q"][b].astype(np.float32).T)
        xk = np.ascontiguousarray(inp["encodings_for_k"][b].astype(np.float32).T)
        xv = np.ascontiguousarray(inp["encodings_for_v"][b].astype(np.float32).T)
        mt = (maskT + padd[b][:, None]).astype(np.float32)
        for g in range(G):
            gs = slice(DG * g, DG * (g + 1))
            in_maps.append({
                "xqt": xq, "xkt": xk, "xvt": xv,
                "wq": np.ascontiguousarray(wq_s[:, gs]),
                "wk": np.ascontiguousarray(inp["Wk"].astype(np.float32)[:, gs]),
                "wv": np.ascontiguousarray(inp["Wv"].astype(np.float32)[:, gs]),
                "bq": np.ascontiguousarray(bq_s[gs].reshape(NJ, 128).T),
                "bk": np.ascontiguousarray(
                    inp["bk"].astype(np.float32)[gs].reshape(NJ, 128).T),
                "bv": np.ascontiguousarray(inp["bv"].astype(np.float32)[gs]),
                "wo": np.ascontiguousarray(inp["Wo"].astype(np.float32)[gs, :]),
                "maskt": mt,
            })
    return in_maps


def kernel(**inputs):
    global last_result
    import os
    from concourse.bass_utils import run_bass_kernel_spmd

    inp = {k: np.asarray(v) for k, v in inputs.items()}
    causal = _is_causal(inp["attn_mask"].astype(np.float32))
    trace = bool(os.environ.get("KBENCH_TRACE"))

    # causal fast path, one retry for transient failures, then the
    # generic-mask fallback; non-finite output counts as failure
    variants = [True, True, False] if causal else [False, False]
    last_exc = None
    for v in variants:
        try:
            nc = _get_nc(v)
            in_maps = (_causal_in_maps if v else _generic_in_maps)(inp)
            res = run_bass_kernel_spmd(
                nc, in_maps, list(range(NCORES)), trace=trace)
            out = np.empty((B, L, D), np.float32)
            for b in range(B):
                out[b] = (res.results[2 * b]["out"]
                          + res.results[2 * b + 1]["out"])
            if not np.isfinite(out).all():
                raise RuntimeError("non-finite output")
            last_result = res
            return out
        except Exception as e:
            last_exc = e
    raise last_exc


# revision 52
# speedup vs baseline: 1.0007x; 1.0007x over previous
"""Multi-head attention (B=4, L=2048, D=1024, H=16) on 8 trn2 NeuronCores.

Sharding: core c = 2*b + g handles batch b and head-group g (8 heads = 512 dims).
Each core computes Q/K/V projections for its group, attention for its 8 heads,
and a partial output projection ctx_g @ Wo[g*512:(g+1)*512, :].  The host sums
the two group partials per batch.

Causal fast path (bf16 PE inputs; ~223.9us tile-cost-model span per core):
  QT, KT  : (512, 2048) feature-major bf16 (4 tiles of (128, L), 2 heads/tile)
  V       : 16 token tiles (128, 8*65) bf16; col 64 of each head = ones column
            that accumulates the softmax denominator Z during the ctx matmul
  scores  : per (head-pair, key-tile) a paired PSUM tile (128, 2, 512) holding
            both heads' score blocks; diagonal key-tiles are column-trimmed to
            the causally-live range and get a (128,2,128) triangular mask add
            on DVE; ONE exp per pair on ACT (bias = padd mask per key)
  ctx     : token-major accumulation — out (128 tok, 65) per 128-query chunk
            (moving dim = 65 features, so ctx matmul cost is ~0.5x of the
            feature-major form); Z lands as a per-partition column, so the
            softmax division is a plain DVE tensor_scalar (no broadcast);
            a tiny PE transpose (identity matmul) restores feature-major
            ctxt for the output projection
  out     : ctxt.T @ Wo chunks into PSUM, DVE/ACT copy, DMA to DRAM f32;
            outproj(t) emission is deferred two blocks and proj(t+1) pieces
            are interleaved between head-pair blocks, so both slot into the
            ACT-bound exp stretches as greedy PE filler; for the final
            block a 2-pass split (jt0-2 into an SBUF partial early, jt3 +
            identity-matmul fold-in late) shortens the tail
  softmax : no max-subtraction (scores are O(3); masked entries hit exp(-1e9)=0)
  x DMAs ride the gpsimd queue, weights/outputs the sync queue, so trigger
  latencies overlap; startup interleaves weight and x chunks per use order.

Non-causal fallback: generic-mask fp32r variant (mask supplied as data).
"""

import sys

if "/opt/trn_rl_repo" not in sys.path:
    sys.path.insert(0, "/opt/trn_rl_repo")

import numpy as np

B, L, D, H = 4, 2048, 1024, 16
G = 2                # head-groups == cores per batch
DG = D // G          # 512 dims per group
HG = H // G          # 8 heads per group
DH = D // H          # 64
NCORES = B * G
NT = L // 512        # query 512-blocks
NKT = L // 128       # key 128-tiles
ND = D // 128        # contraction chunks over input dim
NJ = DG // 128       # dcol tiles per group (2 heads each)

MM_DTYPE = "bfloat16"

_cache = {}


def _build_causal():
    import concourse.bass as bass
    import concourse.tile as tile
    from concourse import bacc, mybir

    f32 = mybir.dt.float32
    f32r = mybir.dt.float32r
    bf16 = mybir.dt.bfloat16
    AF = mybir.ActivationFunctionType

    nc = bacc.Bacc("TRN2")

    xq_d = nc.dram_tensor("xqt", [D, L], bf16, kind="ExternalInput")
    xk_d = nc.dram_tensor("xkt", [D, L], bf16, kind="ExternalInput")
    xv_d = nc.dram_tensor("xvt", [D, L], bf16, kind="ExternalInput")
    wq_d = nc.dram_tensor("wq", [D, DG], bf16, kind="ExternalInput")
    wk_d = nc.dram_tensor("wk", [D, DG], bf16, kind="ExternalInput")
    wv_d = nc.dram_tensor("wv", [D, DG], bf16, kind="ExternalInput")
    bq_d = nc.dram_tensor("bq", [128, NJ], f32, kind="ExternalInput")
    bk_d = nc.dram_tensor("bk", [128, NJ], f32, kind="ExternalInput")
    bv_d = nc.dram_tensor("bv", [DG], f32, kind="ExternalInput")
    wo_d = nc.dram_tensor("wo", [DG, D], bf16, kind="ExternalInput")
    dtri_d = nc.dram_tensor("dtri", [128, 128], f32, kind="ExternalInput")
    pdk_d = nc.dram_tensor("paddk", [128, NKT], f32, kind="ExternalInput")
    id_d = nc.dram_tensor("ident", [128, 128], bf16, kind="ExternalInput")
    out_d = nc.dram_tensor("out", [L, D], f32, kind="ExternalOutput")

    with tile.TileContext(nc) as tc, (
        __import__("contextlib").ExitStack()) as ctx:
        ep = ctx.enter_context
        wpool = ep(tc.tile_pool(name="persist", bufs=1))
        qkpool = ep(tc.tile_pool(name="qk", bufs=1))
        vpool = ep(tc.tile_pool(name="vaug", bufs=1))
        xpool = ep(tc.tile_pool(name="xin", bufs=1))
        scp = ep(tc.tile_pool(name="scps", bufs=2, space="PSUM"))
        cpp = ep(tc.tile_pool(name="ctxps", bufs=2, space="PSUM"))
        gpp = ep(tc.tile_pool(name="gps", bufs=2, space="PSUM"))
        pepool = ep(tc.tile_pool(name="pexp", bufs=3))
        ctpool = ep(tc.tile_pool(name="ctxt", bufs=1))
        rzpool = ep(tc.tile_pool(name="rz", bufs=2))
        opool = ep(tc.tile_pool(name="outsb", bufs=3))

        # ---- persistent weights/biases; DMA order == transfer order, so
        # issue exactly what the first projections need first.
        wq_bg = wpool.tile([128, ND, DG], bf16, tag="wqb", name="wq_bg")
        wk_bg = wpool.tile([128, ND, DG], bf16, tag="wkb", name="wk_bg")
        wv_bg = wpool.tile([128, ND, DG], bf16, tag="wvb", name="wv_bg")
        bq_sb = wpool.tile([128, NJ], f32, tag="bq")
        bk_sb = wpool.tile([128, NJ], f32, tag="bk")
        bv_sb = wpool.tile([128, DG], f32, tag="bv")
        xt = {}
        for name in ("q", "k", "v"):
            xt[name] = [
                xpool.tile([128, ND, 512], bf16, tag=f"x{name}", name=f"x{name}",
                           bufs=3)
                for _ in range(NT)]

        def dma_w_half(wd, wb, hx):
            nc.sync.dma_start(
                out=wb[:, 4 * hx:4 * (hx + 1), :],
                in_=wd[512 * hx:512 * (hx + 1), :].rearrange(
                    "(i p) n -> p i n", p=128))

        def dma_x_chunk(name, xd, t, i, n):
            ts = slice(512 * t, 512 * (t + 1))
            nc.gpsimd.dma_start(
                out=xt[name][t][:, i:i + n, :],
                in_=xd[128 * i:128 * (i + n), ts].rearrange(
                    "(i p) n -> p i n", p=128))

        def dma_x(name, xd, t):
            nc.gpsimd.dma_start(
                out=xt[name][t],
                in_=xd[:, 512 * t:512 * (t + 1)].rearrange(
                    "(i p) n -> p i n", p=128))

        # startup: interleave weight halves with x chunks so the first
        # projection matmuls start as early as possible
        def dma_w_q(wd, wb, i):
            nc.sync.dma_start(
                out=wb[:, i:i + 2, :],
                in_=wd[128 * i:128 * (i + 2), :].rearrange(
                    "(i p) n -> p i n", p=128))

        def dma_w_1(wd, wb, i):
            nc.sync.dma_start(
                out=wb[:, i:i + 1, :],
                in_=wd[128 * i:128 * (i + 1), :].rearrange(
                    "(i p) n -> p i n", p=128))

        dma_w_1(wq_d, wq_bg, 0)
        dma_x_chunk("q", xq_d, 0, 0, 1)
        dma_w_1(wq_d, wq_bg, 1)
        dma_x_chunk("q", xq_d, 0, 1, 1)
        dma_w_q(wq_d, wq_bg, 2)
        dma_x_chunk("q", xq_d, 0, 2, 2)
        dma_w_half(wq_d, wq_bg, 1)
        dma_x_chunk("q", xq_d, 0, 4, 2)
        dma_x_chunk("q", xq_d, 0, 6, 2)
        nc.sync.dma_start(out=bq_sb, in_=bq_d[:, :])
        dma_w_half(wk_d, wk_bg, 0)
        dma_x_chunk("k", xk_d, 0, 0, 2)
        dma_x_chunk("k", xk_d, 0, 2, 2)
        dma_w_half(wk_d, wk_bg, 1)
        dma_x_chunk("k", xk_d, 0, 4, 2)
        dma_x_chunk("k", xk_d, 0, 6, 2)
        nc.sync.dma_start(out=bk_sb, in_=bk_d[:, :])

        dtri = wpool.tile([128, 128], f32, tag="dtri")
        nc.sync.dma_start(out=dtri, in_=dtri_d[:, :])
        pdk_sb = wpool.tile([128, NKT], f32, tag="pdk")
        nc.sync.dma_start(out=pdk_sb, in_=pdk_d[:, :])

        dma_w_half(wv_d, wv_bg, 0)
        dma_x_chunk("v", xv_d, 0, 0, 4)
        dma_w_half(wv_d, wv_bg, 1)
        bv_ap = bv_d[:]
        bv_bcast = bass.AP(
            tensor=bv_ap.tensor, offset=bv_ap.offset,
            ap=[[0, 128]] + list(bv_ap.ap))
        nc.sync.dma_start(out=bv_sb, in_=bv_bcast)
        dma_x_chunk("v", xv_d, 0, 4, 4)

        ident = wpool.tile([128, 128], bf16, tag="ident")
        nc.sync.dma_start(out=ident, in_=id_d[:, :])

        wq_sb = [wq_bg[:, i, :] for i in range(ND)]
        wk_sb = [wk_bg[:, i, :] for i in range(ND)]
        wv_sb = [wv_bg[:, i, :] for i in range(ND)]

        # triangular mask broadcast over the head-pair dim: [128, 2, 128]
        dt_ap = dtri[:, :]
        dtri2 = bass.AP(
            tensor=dt_ap.tensor, offset=dt_ap.offset,
            ap=[dt_ap.ap[0], [0, 2], dt_ap.ap[1]])

        qt_sb = [qkpool.tile([128, L], bf16, tag=f"qt{j}", name="qt_sb")
                 for j in range(NJ)]
        kt_sb = [qkpool.tile([128, L], bf16, tag=f"kt{j}", name="kt_sb")
                 for j in range(NJ)]
        vaug = [vpool.tile([128, HG * 65], bf16, tag=f"va{k}", name="vaug")
                for k in range(NKT)]
        for kt in range(NKT):
            v3 = vaug[kt].rearrange("p (h d) -> p h d", h=HG)
            nc.vector.memset(v3[:, :, 64:65], 1.0)


        wo_bg = wpool.tile([128, NJ, D], bf16, tag="wob", name="wo_bg")
        wo_sb = [wo_bg[:, j, :] for j in range(NJ)]

        def emit_proj(t, only=None):
            ts = slice(512 * t, 512 * (t + 1))
            for name, w_sb, b_sb, dest in (
                ("q", wq_sb, bq_sb, qt_sb),
                ("k", wk_sb, bk_sb, kt_sb),
                ("v", wv_sb, None, None),
            ):
                if only is not None and name != only:
                    continue
                xts = xt[name][t]
                if dest is not None:  # Q/K: feature-major output
                    for j in range(NJ):
                        ps = gpp.tile([128, 512], f32, tag="gp")
                        for i in range(ND):
                            nc.tensor.matmul(
                                out=ps,
                                lhsT=w_sb[i][:, 128 * j:128 * (j + 1)],
                                rhs=xts[:, i, :],
                                start=(i == 0), stop=(i == ND - 1))
                        nc.vector.tensor_scalar_add(
                            out=dest[j][:, ts], in0=ps,
                            scalar1=b_sb[:, j:j + 1])
                else:  # V: token-major output, bv add fused in copy-out
                    for s in range(4):
                        ps = gpp.tile([128, 512], f32, tag="gp")
                        for i in range(ND):
                            nc.tensor.matmul(
                                out=ps,
                                lhsT=xts[:, i, 128 * s:128 * (s + 1)],
                                rhs=wv_sb[i],
                                start=(i == 0), stop=(i == ND - 1))
                        kt = 4 * t + s
                        v3 = vaug[kt].rearrange("p (h d) -> p h d", h=HG)
                        nc.vector.tensor_add(
                            v3[:, :, 0:64],
                            ps.rearrange("p (h d) -> p h d", h=HG),
                            bv_sb.rearrange("p (h d) -> p h d", h=HG))

        def emit_attn(t, fillers=()):
            qs0 = 512 * t
            nkt_t = 4 * t + 4
            ctxt = [ctpool.tile([128, 512], bf16, tag=f"ct{j}", name="ctxt",
                                bufs=4) for j in range(NJ)]
            for hp in range(NJ):
                if hp < len(fillers) and fillers[hp] is not None:
                    fillers[hp]()
                # token-major ctx accumulators: bank X holds query chunks
                # (2X, 2X+1); cols h*65+64 accumulate the softmax denom Z
                # (padded to a full 2048B bank so matmuls stay bank-contained)
                cu = [cpp.tile([128, 2, 256], f32, tag="cu", name="cu",
                               bufs=2) for _ in range(2)]
                for kt in range(nkt_t):
                    ks = slice(128 * kt, 128 * (kt + 1))
                    j = kt - 4 * t  # >= 0 on diagonal tiles
                    o = 128 * j if j >= 0 else 0
                    pair = scp.tile([128, 2, 512], f32, tag="sc", name="scores")
                    for half in range(2):
                        ro = 64 * half
                        nc.tensor.matmul(
                            out=pair[:, half, o:512],
                            lhsT=kt_sb[hp][ro:ro + 64, ks],
                            rhs=qt_sb[hp][ro:ro + 64, qs0 + o:qs0 + 512],
                            start=True, stop=True)
                    if j >= 0:
                        nc.vector.tensor_add(
                            pair[:, :, o:o + 128], pair[:, :, o:o + 128], dtri2)
                    pe = pepool.tile([128, 2, 512], bf16, tag="pe", name="pexp",
                                     bufs=6)
                    nc.scalar.activation(
                        out=pe[:, :, o:512], in_=pair[:, :, o:512],
                        func=AF.Exp, bias=pdk_sb[:, kt:kt + 1])
                    c0 = max(j, 0)
                    for c in range(c0, 4):
                        for half in range(2):
                            h = 2 * hp + half
                            # start zeroes the full 2KB PSUM zero-region, so
                            # only the first matmul into each bank sets it
                            nc.tensor.matmul(
                                out=cu[c // 2][:, c % 2, 65 * half:65 * half + 65],
                                lhsT=pe[:, half, 128 * c:128 * (c + 1)],
                                rhs=vaug[kt][:, 65 * h:65 * (h + 1)],
                                start=(kt == 0 and half == 0 and c % 2 == 0),
                                stop=(kt == 4 * t + 2 * (c // 2) + 1
                                      and c % 2 == 1 and half == 1))
                # normalize: Z sits as per-partition columns; recip + scalar
                # multiply, then PE-transpose back to feature-major ctxt
                cub = ctpool.tile([128, 4, 128], bf16, tag="cub", bufs=2)
                last = (t == NT - 1 and hp == NJ - 1)
                for X in range(2):
                    cu4 = cu[X][:, :, 0:130].rearrange(
                        "p c (h f) -> p c h f", f=65)
                    rz = rzpool.tile([128, 2, 2], f32, tag="rz", bufs=4)
                    nc.vector.reciprocal(out=rz, in_=cu4[:, :, :, 64])
                    for cc in range(2):
                        for half in range(2):
                            co = cub[:, 2 * X + cc, 64 * half:64 * half + 64]
                            if last and half == 1:
                                # ACT is idle at the very end; split the
                                # normalize between both engines
                                nc.scalar.mul(
                                    co, cu4[:, cc, half, 0:64],
                                    rz[:, cc, half:half + 1])
                            else:
                                nc.vector.tensor_scalar_mul(
                                    out=co, in0=cu4[:, cc, half, 0:64],
                                    scalar1=rz[:, cc, half:half + 1])
                tr = cpp.tile([128, 1024], bf16, tag="cu", name="tr")
                for c in range(4):
                    nc.tensor.matmul(
                        out=tr[:, 128 * c:128 * (c + 1)],
                        lhsT=cub[:, c, :],
                        rhs=ident,
                        is_transpose=True,
                        start=(c == 0), stop=(c == 3), skip_group_check=True)
                nc.vector.tensor_copy(out=ctxt[hp], in_=tr[:, 0:512])
            return ctxt

        def emit_outproj(t, ctxt, split=False):
            # split=True: accumulate jt0-2 into an SBUF partial early (fills
            # PE while the last head-pairs' exps drain), only jt3 + add +
            # DMA remain after the final ctxt lands
            parts = {}
            if split:
                for s in range(4):
                    for e in range(2):
                        es = slice(512 * e, 512 * (e + 1))
                        ps = gpp.tile([128, 512], f32, tag="gp")
                        for jt in range(3):
                            nc.tensor.matmul(
                                out=ps,
                                lhsT=ctxt[jt][:, 128 * s:128 * (s + 1)],
                                rhs=wo_sb[jt][:, es],
                                start=(jt == 0), stop=(jt == 2))
                        oa = opool.tile([128, 512], bf16, tag="oa", bufs=8)
                        nc.vector.tensor_copy(out=oa, in_=ps)
                        parts[s, e] = oa
            if split:
                # final pass on sc pair tiles (scores are done): jt3 plus the
                # SBUF partial folded back via identity matmuls, all on PE
                for s in range(4):
                    pr = scp.tile([128, 2, 512], f32, tag="sc", name="opr")
                    for e in range(2):
                        es = slice(512 * e, 512 * (e + 1))
                        # fold the early SBUF partial in FIRST (ready long
                        # before ctxt[3]) so only one matmul remains after
                        # the final head-pair's normalize lands
                        nc.tensor.matmul(
                            out=pr[:, e, :], lhsT=ident, rhs=parts[s, e],
                            start=True, stop=False)
                        nc.tensor.matmul(
                            out=pr[:, e, :],
                            lhsT=ctxt[3][:, 128 * s:128 * (s + 1)],
                            rhs=wo_sb[3][:, es],
                            start=False, stop=True)
                    r0 = 512 * t + 128 * s
                    for e in range(2):
                        es = slice(512 * e, 512 * (e + 1))
                        ob = opool.tile([128, 512], f32, tag="ob", bufs=6)
                        if e == 0:
                            nc.vector.tensor_copy(out=ob, in_=pr[:, e, :])
                        else:
                            nc.scalar.copy(out=ob, in_=pr[:, e, :])
                        if s == 3:
                            # last pair: halve the final DMAs across queues
                            for h in range(2):
                                q = nc.gpsimd if (e + h) % 2 else nc.sync
                                q.dma_start(
                                    out=out_d[r0:r0 + 128,
                                              512 * e + 256 * h:
                                              512 * e + 256 * (h + 1)],
                                    in_=ob[:, 256 * h:256 * (h + 1)])
                        else:
                            q = nc.gpsimd if e == 1 else nc.sync
                            q.dma_start(out=out_d[r0:r0 + 128, es], in_=ob)
            else:
                for s in range(4):
                    for e in range(2):
                        es = slice(512 * e, 512 * (e + 1))
                        ps = gpp.tile([128, 512], f32, tag="gp")
                        for jt in range(NJ):
                            nc.tensor.matmul(
                                out=ps,
                                lhsT=ctxt[jt][:, 128 * s:128 * (s + 1)],
                                rhs=wo_sb[jt][:, es],
                                start=(jt == 0), stop=(jt == NJ - 1))
                        ob = opool.tile([128, 512], f32, tag="ob", bufs=6)
                        if e == 0:
                            nc.vector.tensor_copy(out=ob, in_=ps)
                        else:
                            nc.scalar.copy(out=ob, in_=ps)
                        r0 = 512 * t + 128 * s
                        nc.sync.dma_start(out=out_d[r0:r0 + 128, es], in_=ob)

        emit_proj(0)
        for name, xd in (("q", xq_d), ("k", xk_d), ("v", xv_d)):
            dma_x(name, xd, 1)
        nc.sync.dma_start(
            out=wo_bg, in_=wo_d[:, :].rearrange("(j p) n -> p j n", p=128))
        # outproj(t) is deferred two attention blocks: it becomes greedy PE
        # filler for the ACT-bound stretches of the last attention blocks
        pend = []
        for t in range(NT):
            # proj(t+1) pieces are interleaved between head-pair blocks so
            # their priority slots them into the ACT-bound exp stretches
            if t + 1 < NT:
                fillers = [None] + [
                    (lambda n=n: emit_proj(t + 1, only=n))
                    for n in ("q", "k", "v")]
            else:
                fillers = ()
            pend.append((t, emit_attn(t, fillers)))
            if t + 1 < NT and t + 2 < NT:
                for name, xd in (("q", xq_d), ("k", xk_d), ("v", xv_d)):
                    dma_x(name, xd, t + 2)
            if t >= 2:
                emit_outproj(*pend.pop(0))
        while len(pend) > 1:
            emit_outproj(*pend.pop(0))
        emit_outproj(*pend.pop(0), split=True)

    nc.finalize()
    return nc


def _build_generic():
    """Non-causal fallback: generic additive mask as data, fp32r PE."""
    import concourse.bass as bass
    import concourse.tile as tile
    from concourse import bacc, mybir

    f32 = mybir.dt.float32
    AF = mybir.ActivationFunctionType
    mdt = mybir.dt.float32r

    nc = bacc.Bacc("TRN2")

    xqt = nc.dram_tensor("xqt", [D, L], mdt, kind="ExternalInput")
    xkt = nc.dram_tensor("xkt", [D, L], mdt, kind="ExternalInput")
    xvt = nc.dram_tensor("xvt", [D, L], mdt, kind="ExternalInput")
    wq_d = nc.dram_tensor("wq", [D, DG], mdt, kind="ExternalInput")
    wk_d = nc.dram_tensor("wk", [D, DG], mdt, kind="ExternalInput")
    wv_d = nc.dram_tensor("wv", [D, DG], mdt, kind="ExternalInput")
    bq_d = nc.dram_tensor("bq", [128, NJ], f32, kind="ExternalInput")
    bk_d = nc.dram_tensor("bk", [128, NJ], f32, kind="ExternalInput")
    bv_d = nc.dram_tensor("bv", [DG], f32, kind="ExternalInput")
    wo_d = nc.dram_tensor("wo", [DG, D], mdt, kind="ExternalInput")
    msk_d = nc.dram_tensor("maskt", [L, L], f32, kind="ExternalInput")
    out_d = nc.dram_tensor("out", [L, D], f32, kind="ExternalOutput")

    with tile.TileContext(nc) as tc, (
        __import__("contextlib").ExitStack()) as ctx:
        ep = ctx.enter_context
        wpool = ep(tc.tile_pool(name="persist", bufs=1))
        qkpool = ep(tc.tile_pool(name="qk", bufs=1))
        vpool = ep(tc.tile_pool(name="vaug", bufs=1))
        zdpool = ep(tc.tile_pool(name="zdram", bufs=4, space="DRAM"))
        mmp = ep(tc.tile_pool(name="mm", bufs=4, space="PSUM"))
        ctxp = ep(tc.tile_pool(name="ctxps", bufs=2, space="PSUM"))
        wop = ep(tc.tile_pool(name="wops", bufs=2, space="PSUM"))
        ppool = ep(tc.tile_pool(name="pexp", bufs=3))
        ctpool = ep(tc.tile_pool(name="ctxt", bufs=4))
        cupool = ep(tc.tile_pool(name="ctxu", bufs=2))
        rbpool = ep(tc.tile_pool(name="rbc", bufs=1))
        opool = ep(tc.tile_pool(name="outsb", bufs=2))

        wo_bg = wpool.tile([128, NJ, D], mdt, tag="wob", name="wo_bg")
        nc.sync.dma_start(
            out=wo_bg, in_=wo_d[:, :].rearrange("(j p) n -> p j n", p=128))
        wo_sb = [wo_bg[:, j, :] for j in range(NJ)]
        qt_sb = [qkpool.tile([128, L], mdt, tag=f"qt{j}", name="qt_sb") for j in range(NJ)]
        kt_sb = [qkpool.tile([128, L], mdt, tag=f"kt{j}", name="kt_sb") for j in range(NJ)]
        vaug = [vpool.tile([128, HG * 65], mdt, tag=f"va{k}", name="vaug") for k in range(NKT)]
        ones8 = wpool.tile([128, HG, 1], f32, tag="ones8")
        nc.vector.memset(ones8, 1.0)
        for kt in range(NKT):
            v3 = vaug[kt].rearrange("p (h d) -> p h d", h=HG)
            nc.scalar.copy(out=v3[:, :, 64:65], in_=ones8)

        wq_sb = wk_sb = wv_sb = bq_sb = bk_sb = bv_sb = None

        def setup_w3(w3pool):
            nonlocal wq_sb, wk_sb, wv_sb, bq_sb, bk_sb, bv_sb
            wq_bg = w3pool.tile([128, ND, DG], mdt, tag="wqb", name="wq_bg")
            wk_bg = w3pool.tile([128, ND, DG], mdt, tag="wkb", name="wk_bg")
            wv_bg = w3pool.tile([128, ND, DG], mdt, tag="wvb", name="wv_bg")
            for wd, wb in ((wq_d, wq_bg), (wk_d, wk_bg), (wv_d, wv_bg)):
                nc.sync.dma_start(
                    out=wb, in_=wd[:, :].rearrange("(i p) n -> p i n", p=128))
            wq_sb = [wq_bg[:, i, :] for i in range(ND)]
            wk_sb = [wk_bg[:, i, :] for i in range(ND)]
            wv_sb = [wv_bg[:, i, :] for i in range(ND)]
            bq_sb = w3pool.tile([128, NJ], f32, tag="bq")
            bk_sb = w3pool.tile([128, NJ], f32, tag="bk")
            nc.sync.dma_start(out=bq_sb, in_=bq_d[:, :])
            nc.sync.dma_start(out=bk_sb, in_=bk_d[:, :])
            bv_sb = w3pool.tile([128, DG], f32, tag="bv")
            bv_ap = bv_d[:]
            bv_bcast = bass.AP(
                tensor=bv_ap.tensor, offset=bv_ap.offset,
                ap=[[0, 128]] + list(bv_ap.ap))
            nc.sync.dma_start(out=bv_sb, in_=bv_bcast)

        def emit_proj(t, xpool):
            ts = slice(512 * t, 512 * (t + 1))
            for xd, w_sb, b_sb, dest in (
                (xqt, wq_sb, bq_sb, qt_sb),
                (xkt, wk_sb, bk_sb, kt_sb),
                (xvt, wv_sb, None, None),
            ):
                xts = []
                for hx in range(4):
                    xt_bg = xpool.tile([128, ND // 4, 512], mdt, tag="xt",
                                       name="xt_bg", bufs=3)
                    rs = slice(256 * hx, 256 * (hx + 1))
                    nc.sync.dma_start(
                        out=xt_bg,
                        in_=xd[rs, ts].rearrange("(i p) n -> p i n", p=128))
                    xts.extend(xt_bg[:, i, :] for i in range(ND // 4))
                if dest is not None:  # Q/K: feature-major output
                    for j in range(NJ):
                        ps = mmp.tile([128, 512], f32, tag="mm")
                        for i in range(ND):
                            nc.tensor.matmul(
                                out=ps,
                                lhsT=w_sb[i][:, 128 * j:128 * (j + 1)],
                                rhs=xts[i],
                                start=(i == 0), stop=(i == ND - 1))
                        nc.scalar.activation(
                            out=dest[j][:, ts], in_=ps, func=AF.Identity,
                            bias=b_sb[:, j:j + 1])
                else:  # V: token-major output, bv add fused in copy-out
                    for s in range(4):
                        ps = mmp.tile([128, 512], f32, tag="mm")
                        for i in range(ND):
                            nc.tensor.matmul(
                                out=ps,
                                lhsT=xts[i][:, 128 * s:128 * (s + 1)],
                                rhs=wv_sb[i],
                                start=(i == 0), stop=(i == ND - 1))
                        kt = 4 * t + s
                        v3 = vaug[kt].rearrange("p (h d) -> p h d", h=HG)
                        nc.vector.tensor_add(
                            v3[:, :, 0:64],
                            ps.rearrange("p (h d) -> p h d", h=HG),
                            bv_sb.rearrange("p (h d) -> p h d", h=HG))

        def emit_attn(t, mpool):
            qs = slice(512 * t, 512 * (t + 1))
            msk = []
            for hkt in range(4):
                msk_bg = mpool.tile([128, NKT // 4, 512], f32, tag="msk",
                                    name="msk_bg", bufs=6)
                rs = slice(512 * hkt, 512 * (hkt + 1))
                nc.sync.dma_start(
                    out=msk_bg,
                    in_=msk_d[rs, qs].rearrange("(k p) n -> p k n", p=128))
                msk.extend(msk_bg[:, kt, :] for kt in range(NKT // 4))
            ctxt = [ctpool.tile([128, 512], mdt, tag="ct", name="ctxt") for _ in range(NJ)]
            for hp in range(NJ):
                jt = hp
                ctx_ab = [ctxp.tile([65, 512], f32, tag="ctx", name="ctx_ab") for _ in range(2)]
                for kt in range(NKT):
                    ks = slice(128 * kt, 128 * (kt + 1))
                    pexp = []
                    for half in range(2):
                        ro = 64 * half
                        ps = mmp.tile([128, 512], f32, tag="mm")
                        nc.tensor.matmul(
                            out=ps,
                            lhsT=kt_sb[jt][ro:ro + 64, ks],
                            rhs=qt_sb[jt][ro:ro + 64, qs],
                            start=True, stop=True)
                        nc.vector.tensor_add(ps, ps, msk[kt])
                        pe = ppool.tile([128, 512], mdt, tag="pexp")
                        nc.scalar.activation(out=pe, in_=ps, func=AF.Exp, bias=0.0)
                        pexp.append(pe)
                    for half in range(2):
                        h = 2 * hp + half
                        nc.tensor.matmul(
                            out=ctx_ab[half],
                            lhsT=vaug[kt][:, 65 * h:65 * (h + 1)],
                            rhs=pexp[half],
                            start=(kt == 0), stop=(kt == NKT - 1))
                for half in range(2):
                    ro = 64 * half
                    cu = cupool.tile([65, 512], f32, tag="cu")
                    nc.vector.tensor_copy(out=cu, in_=ctx_ab[half])
                    nc.vector.reciprocal(out=cu[64:65, :], in_=cu[64:65, :])
                    zd = zdpool.tile([1, 512], f32, tag="zd", name="zd")
                    nc.sync.dma_start(out=zd, in_=cu[64:65, :])
                    zrow = zd[0, :]
                    rb_src = bass.AP(
                        tensor=zrow.tensor, offset=zrow.offset,
                        ap=[[0, 64]] + list(zrow.ap))
                    rb = rbpool.tile([64, 512], f32, tag="rb")
                    nc.sync.dma_start(out=rb, in_=rb_src)
                    nc.vector.tensor_mul(
                        ctxt[jt][ro:ro + 64, :], cu[0:64, :], rb)
            for s in range(4):
                for e in range(2):
                    es = slice(512 * e, 512 * (e + 1))
                    ps = wop.tile([128, 512], f32, tag="wo")
                    for jt in range(NJ):
                        nc.tensor.matmul(
                            out=ps,
                            lhsT=ctxt[jt][:, 128 * s:128 * (s + 1)],
                            rhs=wo_sb[jt][:, es],
                            start=(jt == 0), stop=(jt == NJ - 1))
                    ob = opool.tile([128, 512], f32, tag="ob")
                    nc.vector.tensor_copy(out=ob, in_=ps)
                    r0 = 512 * t + 128 * s
                    nc.sync.dma_start(out=out_d[r0:r0 + 128, es], in_=ob)

        with (
            tc.tile_pool(name="w3", bufs=1) as w3pool,
            tc.tile_pool(name="xin", bufs=1) as xpool,
        ):
            setup_w3(w3pool)
            for t in range(NT):
                emit_proj(t, xpool)
        mpool = ep(tc.tile_pool(name="msk", bufs=1))
        for t in range(NT):
            emit_attn(t, mpool)

    nc.finalize()
    return nc


def _get_nc(causal):
    if causal not in _cache:
        _cache[causal] = _build_causal() if causal else _build_generic()
    return _cache[causal]


last_result = None


def _is_causal(attn_mask):
    tri = np.tril(np.ones((L, L), bool))
    expect = np.where(tri, np.float32(0.0), np.float32(-1e9))
    return np.array_equal(attn_mask, expect)


def _causal_in_maps(inp):
    from ml_dtypes import bfloat16

    scale = 1.0 / np.sqrt(np.float32(DH))
    wq_s = (inp["Wq"].astype(np.float32) * scale).astype(bfloat16)
    bq_s = (inp["bq"].astype(np.float32) * scale)
    wk_s = inp["Wk"].astype(np.float32).astype(bfloat16)
    wv_s = inp["Wv"].astype(np.float32).astype(bfloat16)
    wo_s = inp["Wo"].astype(np.float32).astype(bfloat16)
    padd = inp["padd_mask"].astype(np.float32)

    kk = np.arange(128)[:, None]
    qq = np.arange(128)[None, :]
    dtri = np.where(qq >= kk, np.float32(0.0),
                    np.float32(-1e9)).astype(np.float32)
    ident = np.eye(128, dtype=np.float32).astype(bfloat16)

    in_maps = []
    for b in range(B):
        xq = np.ascontiguousarray(
            inp["encodings_for_q"][b].astype(np.float32).T).astype(bfloat16)
        xk = np.ascontiguousarray(
            inp["encodings_for_k"][b].astype(np.float32).T).astype(bfloat16)
        xv = np.ascontiguousarray(
            inp["encodings_for_v"][b].astype(np.float32).T).astype(bfloat16)
        pdk = np.ascontiguousarray(padd[b].reshape(NKT, 128).T)
        for g in range(G):
            gs = slice(DG * g, DG * (g + 1))
            in_maps.append({
                "xqt": xq, "xkt": xk, "xvt": xv,
                "wq": np.ascontiguousarray(wq_s[:, gs]),
                "wk": np.ascontiguousarray(wk_s[:, gs]),
                "wv": np.ascontiguousarray(wv_s[:, gs]),
                "bq": np.ascontiguousarray(bq_s[gs].reshape(NJ, 128).T),
                "bk": np.ascontiguousarray(
                    inp["bk"].astype(np.float32)[gs].reshape(NJ, 128).T),
                "bv": np.ascontiguousarray(inp["bv"].astype(np.float32)[gs]),
                "wo": np.ascontiguousarray(wo_s[gs, :]),
                "dtri": dtri,
                "paddk": pdk,
                "ident": ident,
            })
    return in_maps


def _generic_in_maps(inp):
    scale = 1.0 / np.sqrt(np.float32(DH))
    wq_s = (inp["Wq"] * scale).astype(np.float32)
    bq_s = (inp["bq"] * scale).astype(np.float32)
    padd = inp["padd_mask"].astype(np.float32)
    maskT = np.ascontiguousarray(inp["attn_mask"].astype(np.float32).T)

    in_maps = []
    for b in range(B):
        xq = np.ascontiguousarray(inp["encodings_for_q"][b].astype(np.float32).T)
        xk = np.ascontiguousarray(inp["encodings_for_k"][b].astype(np.float32).T)
        xv = np.ascontiguousarray(inp["encodings_for_v"][b].astype(np.float32).T)
        mt = (maskT + padd[b][:, None]).astype(np.float32)
        for g in range(G):
            gs = slice(DG * g, DG * (g + 1))
            in_maps.append({
                "xqt": xq, "xkt": xk, "xvt": xv,
                "wq": np.ascontiguousarray(wq_s[:, gs]),
                "wk": np.ascontiguousarray(inp["Wk"].astype(np.float32)[:, gs]),
                "wv": np.ascontiguousarray(inp["Wv"].astype(np.float32)[:, gs]),
                "bq": np.ascontiguousarray(bq_s[gs].reshape(NJ, 128).T),
                "bk": np.ascontiguousarray(
                    inp["bk"].astype(np.float32)[gs].reshape(NJ, 128).T),
                "bv": np.ascontiguousarray(inp["bv"].astype(np.float32)[gs]),
                "wo": np.ascontiguousarray(inp["Wo"].astype(np.float32)[gs, :]),
                "maskt": mt,
            })
    return in_maps


def kernel(**inputs):
    global last_result
    import os
    from concourse.bass_utils import run_bass_kernel_spmd

    inp = {k: np.asarray(v) for k, v in inputs.items()}
    causal = _is_causal(inp["attn_mask"].astype(np.float32))
    trace = bool(os.environ.get("KBENCH_TRACE"))

    # causal fast path, one retry for transient failures, then the
    # generic-mask fallback; non-finite output counts as failure
    variants = [True, True, False] if causal else [False, False]
    last_exc = None
    for v in variants:
        try:
            nc = _get_nc(v)
            in_maps = (_causal_in_maps if v else _generic_in_maps)(inp)
            res = run_bass_kernel_spmd(
                nc, in_maps, list(range(NCORES)), trace=trace)
            out = np.empty((B, L, D), np.float32)
            for b in range(B):
                out[b] = (res.results[2 * b]["out"]
                          + res.results[2 * b + 1]["out"])
            if not np.isfinite(out).all():
                raise RuntimeError("non-finite output")
            last_result = res
            return out
        except Exception as e:
            last_exc = e
    raise last_exc


# revision 55
# speedup vs baseline: 1.0117x; 1.0110x over previous
"""Multi-head attention (B=4, L=2048, D=1024, H=16) on 8 trn2 NeuronCores.

Sharding: core c = 2*b + g handles batch b and head-group g (8 heads = 512 dims).
Each core computes Q/K/V projections for its group, attention for its 8 heads,
and a partial output projection ctx_g @ Wo[g*512:(g+1)*512, :].  The host sums
the two group partials per batch.

Causal fast path (bf16 PE inputs; ~221.5us tile-cost-model span per core):
  QT, KT  : (512, 2048) feature-major bf16 (4 tiles of (128, L), 2 heads/tile)
  V       : 16 token tiles (128, 8*65) bf16; col 64 of each head = ones column
            that accumulates the softmax denominator Z during the ctx matmul
  scores  : per (head-pair, key-tile) a paired PSUM tile (128, 2, 512) holding
            both heads' score blocks; diagonal key-tiles are column-trimmed to
            the causally-live range and get a (128,2,128) triangular mask add
            on DVE; ONE exp per pair on ACT (bias = padd mask per key)
  ctx     : token-major accumulation — out (128 tok, 65) per 128-query chunk
            (moving dim = 65 features, so ctx matmul cost is ~0.5x of the
            feature-major form); Z lands as a per-partition column, so the
            softmax division is a plain DVE tensor_scalar (no broadcast);
            a tiny PE transpose (identity matmul) restores feature-major
            ctxt for the output projection
  out     : ctxt.T @ Wo chunks into PSUM, DVE/ACT copy, DMA to DRAM f32;
            outproj(t) emission is deferred two blocks and proj(t+1) pieces
            are interleaved between head-pair blocks, so both slot into the
            ACT-bound exp stretches as greedy PE filler; for the final
            block a 2-pass split (jt0-2 into an SBUF partial early, jt3 +
            identity-matmul fold-in late) shortens the tail
  softmax : no max-subtraction (scores are O(3); masked entries hit exp(-1e9)=0)
  x DMAs ride the gpsimd queue, weights/outputs the sync queue, so trigger
  latencies overlap; startup interleaves weight and x chunks per use order.

Non-causal fallback: generic-mask fp32r variant (mask supplied as data).
"""

import sys

if "/opt/trn_rl_repo" not in sys.path:
    sys.path.insert(0, "/opt/trn_rl_repo")

import numpy as np

B, L, D, H = 4, 2048, 1024, 16
G = 2                # head-groups == cores per batch
DG = D // G          # 512 dims per group
HG = H // G          # 8 heads per group
DH = D // H          # 64
NCORES = B * G
NT = L // 512        # query 512-blocks
NKT = L // 128       # key 128-tiles
ND = D // 128        # contraction chunks over input dim
NJ = DG // 128       # dcol tiles per group (2 heads each)

MM_DTYPE = "bfloat16"

_cache = {}


def _build_causal():
    import concourse.bass as bass
    import concourse.tile as tile
    from concourse import bacc, mybir

    f32 = mybir.dt.float32
    f32r = mybir.dt.float32r
    bf16 = mybir.dt.bfloat16
    AF = mybir.ActivationFunctionType

    nc = bacc.Bacc("TRN2")

    xq_d = nc.dram_tensor("xqt", [D, L], bf16, kind="ExternalInput")
    xk_d = nc.dram_tensor("xkt", [D, L], bf16, kind="ExternalInput")
    xv_d = nc.dram_tensor("xvt", [D, L], bf16, kind="ExternalInput")
    wq_d = nc.dram_tensor("wq", [D, DG], bf16, kind="ExternalInput")
    wk_d = nc.dram_tensor("wk", [D, DG], bf16, kind="ExternalInput")
    wv_d = nc.dram_tensor("wv", [D, DG], bf16, kind="ExternalInput")
    bq_d = nc.dram_tensor("bq", [128, NJ], f32, kind="ExternalInput")
    bk_d = nc.dram_tensor("bk", [128, NJ], f32, kind="ExternalInput")
    bv_d = nc.dram_tensor("bv", [DG], f32, kind="ExternalInput")
    wo_d = nc.dram_tensor("wo", [DG, D], bf16, kind="ExternalInput")
    dtri_d = nc.dram_tensor("dtri", [128, 128], f32, kind="ExternalInput")
    pdk_d = nc.dram_tensor("paddk", [128, NKT], f32, kind="ExternalInput")
    id_d = nc.dram_tensor("ident", [128, 128], bf16, kind="ExternalInput")
    out_d = nc.dram_tensor("out", [L, D], f32, kind="ExternalOutput")

    with tile.TileContext(nc) as tc, (
        __import__("contextlib").ExitStack()) as ctx:
        ep = ctx.enter_context
        wpool = ep(tc.tile_pool(name="persist", bufs=1))
        qkpool = ep(tc.tile_pool(name="qk", bufs=1))
        vpool = ep(tc.tile_pool(name="vaug", bufs=1))
        xpool = ep(tc.tile_pool(name="xin", bufs=1))
        scp = ep(tc.tile_pool(name="scps", bufs=2, space="PSUM"))
        cpp = ep(tc.tile_pool(name="ctxps", bufs=2, space="PSUM"))
        gpp = ep(tc.tile_pool(name="gps", bufs=2, space="PSUM"))
        pepool = ep(tc.tile_pool(name="pexp", bufs=3))
        ctpool = ep(tc.tile_pool(name="ctxt", bufs=1))
        rzpool = ep(tc.tile_pool(name="rz", bufs=2))
        opool = ep(tc.tile_pool(name="outsb", bufs=3))

        # ---- persistent weights/biases; DMA order == transfer order, so
        # issue exactly what the first projections need first.
        wq_bg = wpool.tile([128, ND, DG], bf16, tag="wqb", name="wq_bg")
        wk_bg = wpool.tile([128, ND, DG], bf16, tag="wkb", name="wk_bg")
        wv_bg = wpool.tile([128, ND, DG], bf16, tag="wvb", name="wv_bg")
        bq_sb = wpool.tile([128, NJ], f32, tag="bq")
        bk_sb = wpool.tile([128, NJ], f32, tag="bk")
        bv_sb = wpool.tile([128, DG], f32, tag="bv")
        xt = {}
        for name in ("q", "k", "v"):
            xt[name] = [
                xpool.tile([128, ND, 512], bf16, tag=f"x{name}", name=f"x{name}",
                           bufs=3)
                for _ in range(NT)]

        def dma_w_half(wd, wb, hx):
            nc.sync.dma_start(
                out=wb[:, 4 * hx:4 * (hx + 1), :],
                in_=wd[512 * hx:512 * (hx + 1), :].rearrange(
                    "(i p) n -> p i n", p=128))

        def dma_x_chunk(name, xd, t, i, n):
            ts = slice(512 * t, 512 * (t + 1))
            nc.gpsimd.dma_start(
                out=xt[name][t][:, i:i + n, :],
                in_=xd[128 * i:128 * (i + n), ts].rearrange(
                    "(i p) n -> p i n", p=128))

        def dma_x(name, xd, t):
            nc.gpsimd.dma_start(
                out=xt[name][t],
                in_=xd[:, 512 * t:512 * (t + 1)].rearrange(
                    "(i p) n -> p i n", p=128))

        # startup: interleave weight halves with x chunks so the first
        # projection matmuls start as early as possible
        def dma_w_q(wd, wb, i):
            nc.sync.dma_start(
                out=wb[:, i:i + 2, :],
                in_=wd[128 * i:128 * (i + 2), :].rearrange(
                    "(i p) n -> p i n", p=128))

        def dma_w_1(wd, wb, i):
            nc.sync.dma_start(
                out=wb[:, i:i + 1, :],
                in_=wd[128 * i:128 * (i + 1), :].rearrange(
                    "(i p) n -> p i n", p=128))

        dma_w_1(wq_d, wq_bg, 0)
        dma_x_chunk("q", xq_d, 0, 0, 1)
        dma_w_1(wq_d, wq_bg, 1)
        dma_x_chunk("q", xq_d, 0, 1, 1)
        dma_w_q(wq_d, wq_bg, 2)
        dma_x_chunk("q", xq_d, 0, 2, 2)
        dma_w_half(wq_d, wq_bg, 1)
        dma_x_chunk("q", xq_d, 0, 4, 2)
        dma_x_chunk("q", xq_d, 0, 6, 2)
        nc.sync.dma_start(out=bq_sb, in_=bq_d[:, :])
        dma_w_half(wk_d, wk_bg, 0)
        dma_x_chunk("k", xk_d, 0, 0, 2)
        dma_x_chunk("k", xk_d, 0, 2, 2)
        dma_w_half(wk_d, wk_bg, 1)
        dma_x_chunk("k", xk_d, 0, 4, 2)
        dma_x_chunk("k", xk_d, 0, 6, 2)
        nc.sync.dma_start(out=bk_sb, in_=bk_d[:, :])

        dtri = wpool.tile([128, 128], f32, tag="dtri")
        nc.sync.dma_start(out=dtri, in_=dtri_d[:, :])
        pdk_sb = wpool.tile([128, NKT], f32, tag="pdk")
        nc.sync.dma_start(out=pdk_sb, in_=pdk_d[:, :])

        dma_w_half(wv_d, wv_bg, 0)
        dma_x_chunk("v", xv_d, 0, 0, 4)
        dma_w_half(wv_d, wv_bg, 1)
        bv_ap = bv_d[:]
        bv_bcast = bass.AP(
            tensor=bv_ap.tensor, offset=bv_ap.offset,
            ap=[[0, 128]] + list(bv_ap.ap))
        nc.sync.dma_start(out=bv_sb, in_=bv_bcast)
        dma_x_chunk("v", xv_d, 0, 4, 4)

        ident = wpool.tile([128, 128], bf16, tag="ident")
        nc.sync.dma_start(out=ident, in_=id_d[:, :])

        wq_sb = [wq_bg[:, i, :] for i in range(ND)]
        wk_sb = [wk_bg[:, i, :] for i in range(ND)]
        wv_sb = [wv_bg[:, i, :] for i in range(ND)]

        # triangular mask broadcast over the head-pair dim: [128, 2, 128]
        dt_ap = dtri[:, :]
        dtri2 = bass.AP(
            tensor=dt_ap.tensor, offset=dt_ap.offset,
            ap=[dt_ap.ap[0], [0, 2], dt_ap.ap[1]])

        qt_sb = [qkpool.tile([128, L], bf16, tag=f"qt{j}", name="qt_sb")
                 for j in range(NJ)]
        kt_sb = [qkpool.tile([128, L], bf16, tag=f"kt{j}", name="kt_sb")
                 for j in range(NJ)]
        vaug = [vpool.tile([128, HG * 65], bf16, tag=f"va{k}", name="vaug")
                for k in range(NKT)]
        for kt in range(NKT):
            v3 = vaug[kt].rearrange("p (h d) -> p h d", h=HG)
            nc.vector.memset(v3[:, :, 64:65], 1.0)


        wo_bg = wpool.tile([128, NJ, D], bf16, tag="wob", name="wo_bg")
        wo_sb = [wo_bg[:, j, :] for j in range(NJ)]

        def emit_proj(t, only=None):
            ts = slice(512 * t, 512 * (t + 1))
            for name, w_sb, b_sb, dest in (
                ("q", wq_sb, bq_sb, qt_sb),
                ("k", wk_sb, bk_sb, kt_sb),
                ("v", wv_sb, None, None),
            ):
                if only is not None and name != only:
                    continue
                xts = xt[name][t]
                if dest is not None:  # Q/K: feature-major output
                    for j in range(NJ):
                        ps = gpp.tile([128, 512], f32, tag="gp")
                        for i in range(ND):
                            nc.tensor.matmul(
                                out=ps,
                                lhsT=w_sb[i][:, 128 * j:128 * (j + 1)],
                                rhs=xts[:, i, :],
                                start=(i == 0), stop=(i == ND - 1))
                        nc.vector.tensor_scalar_add(
                            out=dest[j][:, ts], in0=ps,
                            scalar1=b_sb[:, j:j + 1])
                else:  # V: token-major output, bv add fused in copy-out
                    for s in range(4):
                        ps = gpp.tile([128, 512], f32, tag="gp")
                        for i in range(ND):
                            nc.tensor.matmul(
                                out=ps,
                                lhsT=xts[:, i, 128 * s:128 * (s + 1)],
                                rhs=wv_sb[i],
                                start=(i == 0), stop=(i == ND - 1))
                        kt = 4 * t + s
                        v3 = vaug[kt].rearrange("p (h d) -> p h d", h=HG)
                        nc.vector.tensor_add(
                            v3[:, :, 0:64],
                            ps.rearrange("p (h d) -> p h d", h=HG),
                            bv_sb.rearrange("p (h d) -> p h d", h=HG))

        def emit_attn(t, fillers=()):
            qs0 = 512 * t
            nkt_t = 4 * t + 4
            ctxt = [ctpool.tile([128, 512], bf16, tag=f"ct{j}", name="ctxt",
                                bufs=4) for j in range(NJ)]
            for hp in range(NJ):
                if hp < len(fillers) and fillers[hp] is not None:
                    fillers[hp]()
                # token-major ctx accumulators: bank X holds query chunks
                # (2X, 2X+1); cols h*65+64 accumulate the softmax denom Z
                # (padded to a full 2048B bank so matmuls stay bank-contained)
                cu = [cpp.tile([128, 2, 256], f32, tag="cu", name="cu",
                               bufs=2) for _ in range(2)]
                for kt in range(nkt_t):
                    ks = slice(128 * kt, 128 * (kt + 1))
                    j = kt - 4 * t  # >= 0 on diagonal tiles
                    o = 128 * j if j >= 0 else 0
                    pair = scp.tile([128, 2, 512], f32, tag="sc", name="scores")
                    for half in range(2):
                        ro = 64 * half
                        nc.tensor.matmul(
                            out=pair[:, half, o:512],
                            lhsT=kt_sb[hp][ro:ro + 64, ks],
                            rhs=qt_sb[hp][ro:ro + 64, qs0 + o:qs0 + 512],
                            start=True, stop=True)
                    if j >= 0:
                        nc.vector.tensor_add(
                            pair[:, :, o:o + 128], pair[:, :, o:o + 128], dtri2)
                    pe = pepool.tile([128, 2, 512], bf16, tag="pe", name="pexp",
                                     bufs=6)
                    nc.scalar.activation(
                        out=pe[:, :, o:512], in_=pair[:, :, o:512],
                        func=AF.Exp, bias=pdk_sb[:, kt:kt + 1])
                    c0 = max(j, 0)
                    for c in range(c0, 4):
                        for half in range(2):
                            h = 2 * hp + half
                            # start zeroes the full 2KB PSUM zero-region, so
                            # only the first matmul into each bank sets it
                            nc.tensor.matmul(
                                out=cu[c // 2][:, c % 2, 65 * half:65 * half + 65],
                                lhsT=pe[:, half, 128 * c:128 * (c + 1)],
                                rhs=vaug[kt][:, 65 * h:65 * (h + 1)],
                                start=(kt == 0 and half == 0 and c % 2 == 0),
                                stop=(kt == 4 * t + 2 * (c // 2) + 1
                                      and c % 2 == 1 and half == 1))
                # normalize: Z sits as per-partition columns; recip + scalar
                # multiply, then PE-transpose back to feature-major ctxt
                cub = ctpool.tile([128, 4, 128], bf16, tag="cub", bufs=2)
                last = (t == NT - 1 and hp == NJ - 1)
                for X in range(2):
                    cu4 = cu[X][:, :, 0:130].rearrange(
                        "p c (h f) -> p c h f", f=65)
                    rz = rzpool.tile([128, 2, 2], f32, tag="rz", bufs=4)
                    nc.vector.reciprocal(out=rz, in_=cu4[:, :, :, 64])
                    for cc in range(2):
                        for half in range(2):
                            co = cub[:, 2 * X + cc, 64 * half:64 * half + 64]
                            if last and half == 1:
                                # ACT is idle at the very end; split the
                                # normalize between both engines
                                nc.scalar.mul(
                                    co, cu4[:, cc, half, 0:64],
                                    rz[:, cc, half:half + 1])
                            else:
                                nc.vector.tensor_scalar_mul(
                                    out=co, in0=cu4[:, cc, half, 0:64],
                                    scalar1=rz[:, cc, half:half + 1])
                tr = cpp.tile([128, 1024], bf16, tag="cu", name="tr")
                for c in range(4):
                    nc.tensor.matmul(
                        out=tr[:, 128 * c:128 * (c + 1)],
                        lhsT=cub[:, c, :],
                        rhs=ident,
                        is_transpose=True,
                        start=(c == 0), stop=(c == 3), skip_group_check=True)
                if last:
                    # final head-pair: split the drain across DVE and ACT so
                    # the last output-projection pass starts sooner
                    nc.vector.tensor_copy(
                        out=ctxt[hp][:, 0:256], in_=tr[:, 0:256])
                    nc.scalar.copy(
                        out=ctxt[hp][:, 256:512], in_=tr[:, 256:512])
                else:
                    nc.vector.tensor_copy(out=ctxt[hp], in_=tr[:, 0:512])
            return ctxt

        def emit_outproj(t, ctxt, split=False):
            # split=True: accumulate jt0-2 into an SBUF partial early (fills
            # PE while the last head-pairs' exps drain), only jt3 + add +
            # DMA remain after the final ctxt lands
            parts = {}
            if split:
                for s in range(4):
                    for e in range(2):
                        es = slice(512 * e, 512 * (e + 1))
                        ps = gpp.tile([128, 512], f32, tag="gp")
                        for jt in range(3):
                            nc.tensor.matmul(
                                out=ps,
                                lhsT=ctxt[jt][:, 128 * s:128 * (s + 1)],
                                rhs=wo_sb[jt][:, es],
                                start=(jt == 0), stop=(jt == 2))
                        oa = opool.tile([128, 512], bf16, tag="oa", bufs=8)
                        nc.vector.tensor_copy(out=oa, in_=ps)
                        parts[s, e] = oa
            if split:
                # final pass on sc pair tiles (scores are done): jt3 plus the
                # SBUF partial folded back via identity matmuls, all on PE
                for s in range(4):
                    pr = scp.tile([128, 2, 512], f32, tag="sc", name="opr")
                    for e in range(2):
                        es = slice(512 * e, 512 * (e + 1))
                        # fold the early SBUF partial in FIRST (ready long
                        # before ctxt[3]) so only one matmul remains after
                        # the final head-pair's normalize lands
                        nc.tensor.matmul(
                            out=pr[:, e, :], lhsT=ident, rhs=parts[s, e],
                            start=True, stop=False)
                        nc.tensor.matmul(
                            out=pr[:, e, :],
                            lhsT=ctxt[3][:, 128 * s:128 * (s + 1)],
                            rhs=wo_sb[3][:, es],
                            start=False, stop=True)
                    r0 = 512 * t + 128 * s
                    for e in range(2):
                        es = slice(512 * e, 512 * (e + 1))
                        ob = opool.tile([128, 512], f32, tag="ob", bufs=6)
                        if e == 0:
                            nc.vector.tensor_copy(out=ob, in_=pr[:, e, :])
                        else:
                            nc.scalar.copy(out=ob, in_=pr[:, e, :])
                        if s == 3:
                            # last pair: halve the final DMAs across queues
                            for h in range(2):
                                q = nc.gpsimd if (e + h) % 2 else nc.sync
                                q.dma_start(
                                    out=out_d[r0:r0 + 128,
                                              512 * e + 256 * h:
                                              512 * e + 256 * (h + 1)],
                                    in_=ob[:, 256 * h:256 * (h + 1)])
                        else:
                            q = nc.gpsimd if e == 1 else nc.sync
                            q.dma_start(out=out_d[r0:r0 + 128, es], in_=ob)
            else:
                for s in range(4):
                    for e in range(2):
                        es = slice(512 * e, 512 * (e + 1))
                        ps = gpp.tile([128, 512], f32, tag="gp")
                        for jt in range(NJ):
                            nc.tensor.matmul(
                                out=ps,
                                lhsT=ctxt[jt][:, 128 * s:128 * (s + 1)],
                                rhs=wo_sb[jt][:, es],
                                start=(jt == 0), stop=(jt == NJ - 1))
                        # DVE-only drains: these outprojs run as filler in
                        # exp-saturated attention windows, so an ACT drain
                        # would delay the exp stream directly
                        ob = opool.tile([128, 512], f32, tag="ob", bufs=6)
                        nc.vector.tensor_copy(out=ob, in_=ps)
                        r0 = 512 * t + 128 * s
                        nc.sync.dma_start(out=out_d[r0:r0 + 128, es], in_=ob)

        emit_proj(0)
        for name, xd in (("q", xq_d), ("k", xk_d), ("v", xv_d)):
            dma_x(name, xd, 1)
        nc.sync.dma_start(
            out=wo_bg, in_=wo_d[:, :].rearrange("(j p) n -> p j n", p=128))
        # outproj(t) is deferred two attention blocks: it becomes greedy PE
        # filler for the ACT-bound stretches of the last attention blocks
        pend = []
        for t in range(NT):
            # proj(t+1) pieces are interleaved between head-pair blocks so
            # their priority slots them into the ACT-bound exp stretches
            if t + 1 < NT:
                fillers = [None] + [
                    (lambda n=n: emit_proj(t + 1, only=n))
                    for n in ("q", "k", "v")]
            else:
                fillers = ()
            pend.append((t, emit_attn(t, fillers)))
            if t + 1 < NT and t + 2 < NT:
                for name, xd in (("q", xq_d), ("k", xk_d), ("v", xv_d)):
                    dma_x(name, xd, t + 2)
            if t >= 2:
                emit_outproj(*pend.pop(0))
        while len(pend) > 1:
            emit_outproj(*pend.pop(0))
        emit_outproj(*pend.pop(0), split=True)

    nc.finalize()
    return nc


def _build_generic():
    """Non-causal fallback: generic additive mask as data, fp32r PE."""
    import concourse.bass as bass
    import concourse.tile as tile
    from concourse import bacc, mybir

    f32 = mybir.dt.float32
    AF = mybir.ActivationFunctionType
    mdt = mybir.dt.float32r

    nc = bacc.Bacc("TRN2")

    xqt = nc.dram_tensor("xqt", [D, L], mdt, kind="ExternalInput")
    xkt = nc.dram_tensor("xkt", [D, L], mdt, kind="ExternalInput")
    xvt = nc.dram_tensor("xvt", [D, L], mdt, kind="ExternalInput")
    wq_d = nc.dram_tensor("wq", [D, DG], mdt, kind="ExternalInput")
    wk_d = nc.dram_tensor("wk", [D, DG], mdt, kind="ExternalInput")
    wv_d = nc.dram_tensor("wv", [D, DG], mdt, kind="ExternalInput")
    bq_d = nc.dram_tensor("bq", [128, NJ], f32, kind="ExternalInput")
    bk_d = nc.dram_tensor("bk", [128, NJ], f32, kind="ExternalInput")
    bv_d = nc.dram_tensor("bv", [DG], f32, kind="ExternalInput")
    wo_d = nc.dram_tensor("wo", [DG, D], mdt, kind="ExternalInput")
    msk_d = nc.dram_tensor("maskt", [L, L], f32, kind="ExternalInput")
    out_d = nc.dram_tensor("out", [L, D], f32, kind="ExternalOutput")

    with tile.TileContext(nc) as tc, (
        __import__("contextlib").ExitStack()) as ctx:
        ep = ctx.enter_context
        wpool = ep(tc.tile_pool(name="persist", bufs=1))
        qkpool = ep(tc.tile_pool(name="qk", bufs=1))
        vpool = ep(tc.tile_pool(name="vaug", bufs=1))
        zdpool = ep(tc.tile_pool(name="zdram", bufs=4, space="DRAM"))
        mmp = ep(tc.tile_pool(name="mm", bufs=4, space="PSUM"))
        ctxp = ep(tc.tile_pool(name="ctxps", bufs=2, space="PSUM"))
        wop = ep(tc.tile_pool(name="wops", bufs=2, space="PSUM"))
        ppool = ep(tc.tile_pool(name="pexp", bufs=3))
        ctpool = ep(tc.tile_pool(name="ctxt", bufs=4))
        cupool = ep(tc.tile_pool(name="ctxu", bufs=2))
        rbpool = ep(tc.tile_pool(name="rbc", bufs=1))
        opool = ep(tc.tile_pool(name="outsb", bufs=2))

        wo_bg = wpool.tile([128, NJ, D], mdt, tag="wob", name="wo_bg")
        nc.sync.dma_start(
            out=wo_bg, in_=wo_d[:, :].rearrange("(j p) n -> p j n", p=128))
        wo_sb = [wo_bg[:, j, :] for j in range(NJ)]
        qt_sb = [qkpool.tile([128, L], mdt, tag=f"qt{j}", name="qt_sb") for j in range(NJ)]
        kt_sb = [qkpool.tile([128, L], mdt, tag=f"kt{j}", name="kt_sb") for j in range(NJ)]
        vaug = [vpool.tile([128, HG * 65], mdt, tag=f"va{k}", name="vaug") for k in range(NKT)]
        ones8 = wpool.tile([128, HG, 1], f32, tag="ones8")
        nc.vector.memset(ones8, 1.0)
        for kt in range(NKT):
            v3 = vaug[kt].rearrange("p (h d) -> p h d", h=HG)
            nc.scalar.copy(out=v3[:, :, 64:65], in_=ones8)

        wq_sb = wk_sb = wv_sb = bq_sb = bk_sb = bv_sb = None

        def setup_w3(w3pool):
            nonlocal wq_sb, wk_sb, wv_sb, bq_sb, bk_sb, bv_sb
            wq_bg = w3pool.tile([128, ND, DG], mdt, tag="wqb", name="wq_bg")
            wk_bg = w3pool.tile([128, ND, DG], mdt, tag="wkb", name="wk_bg")
            wv_bg = w3pool.tile([128, ND, DG], mdt, tag="wvb", name="wv_bg")
            for wd, wb in ((wq_d, wq_bg), (wk_d, wk_bg), (wv_d, wv_bg)):
                nc.sync.dma_start(
                    out=wb, in_=wd[:, :].rearrange("(i p) n -> p i n", p=128))
            wq_sb = [wq_bg[:, i, :] for i in range(ND)]
            wk_sb = [wk_bg[:, i, :] for i in range(ND)]
            wv_sb = [wv_bg[:, i, :] for i in range(ND)]
            bq_sb = w3pool.tile([128, NJ], f32, tag="bq")
            bk_sb = w3pool.tile([128, NJ], f32, tag="bk")
            nc.sync.dma_start(out=bq_sb, in_=bq_d[:, :])
            nc.sync.dma_start(out=bk_sb, in_=bk_d[:, :])
            bv_sb = w3pool.tile([128, DG], f32, tag="bv")
            bv_ap = bv_d[:]
            bv_bcast = bass.AP(
                tensor=bv_ap.tensor, offset=bv_ap.offset,
                ap=[[0, 128]] + list(bv_ap.ap))
            nc.sync.dma_start(out=bv_sb, in_=bv_bcast)

        def emit_proj(t, xpool):
            ts = slice(512 * t, 512 * (t + 1))
            for xd, w_sb, b_sb, dest in (
                (xqt, wq_sb, bq_sb, qt_sb),
                (xkt, wk_sb, bk_sb, kt_sb),
                (xvt, wv_sb, None, None),
            ):
                xts = []
                for hx in range(4):
                    xt_bg = xpool.tile([128, ND // 4, 512], mdt, tag="xt",
                                       name="xt_bg", bufs=3)
                    rs = slice(256 * hx, 256 * (hx + 1))
                    nc.sync.dma_start(
                        out=xt_bg,
                        in_=xd[rs, ts].rearrange("(i p) n -> p i n", p=128))
                    xts.extend(xt_bg[:, i, :] for i in range(ND // 4))
                if dest is not None:  # Q/K: feature-major output
                    for j in range(NJ):
                        ps = mmp.tile([128, 512], f32, tag="mm")
                        for i in range(ND):
                            nc.tensor.matmul(
                                out=ps,
                                lhsT=w_sb[i][:, 128 * j:128 * (j + 1)],
                                rhs=xts[i],
                                start=(i == 0), stop=(i == ND - 1))
                        nc.scalar.activation(
                            out=dest[j][:, ts], in_=ps, func=AF.Identity,
                            bias=b_sb[:, j:j + 1])
                else:  # V: token-major output, bv add fused in copy-out
                    for s in range(4):
                        ps = mmp.tile([128, 512], f32, tag="mm")
                        for i in range(ND):
                            nc.tensor.matmul(
                                out=ps,
                                lhsT=xts[i][:, 128 * s:128 * (s + 1)],
                                rhs=wv_sb[i],
                                start=(i == 0), stop=(i == ND - 1))
                        kt = 4 * t + s
                        v3 = vaug[kt].rearrange("p (h d) -> p h d", h=HG)
                        nc.vector.tensor_add(
                            v3[:, :, 0:64],
                            ps.rearrange("p (h d) -> p h d", h=HG),
                            bv_sb.rearrange("p (h d) -> p h d", h=HG))

        def emit_attn(t, mpool):
            qs = slice(512 * t, 512 * (t + 1))
            msk = []
            for hkt in range(4):
                msk_bg = mpool.tile([128, NKT // 4, 512], f32, tag="msk",
                                    name="msk_bg", bufs=6)
                rs = slice(512 * hkt, 512 * (hkt + 1))
                nc.sync.dma_start(
                    out=msk_bg,
                    in_=msk_d[rs, qs].rearrange("(k p) n -> p k n", p=128))
                msk.extend(msk_bg[:, kt, :] for kt in range(NKT // 4))
            ctxt = [ctpool.tile([128, 512], mdt, tag="ct", name="ctxt") for _ in range(NJ)]
            for hp in range(NJ):
                jt = hp
                ctx_ab = [ctxp.tile([65, 512], f32, tag="ctx", name="ctx_ab") for _ in range(2)]
                for kt in range(NKT):
                    ks = slice(128 * kt, 128 * (kt + 1))
                    pexp = []
                    for half in range(2):
                        ro = 64 * half
                        ps = mmp.tile([128, 512], f32, tag="mm")
                        nc.tensor.matmul(
                            out=ps,
                            lhsT=kt_sb[jt][ro:ro + 64, ks],
                            rhs=qt_sb[jt][ro:ro + 64, qs],
                            start=True, stop=True)
                        nc.vector.tensor_add(ps, ps, msk[kt])
                        pe = ppool.tile([128, 512], mdt, tag="pexp")
                        nc.scalar.activation(out=pe, in_=ps, func=AF.Exp, bias=0.0)
                        pexp.append(pe)
                    for half in range(2):
                        h = 2 * hp + half
                        nc.tensor.matmul(
                            out=ctx_ab[half],
                            lhsT=vaug[kt][:, 65 * h:65 * (h + 1)],
                            rhs=pexp[half],
                            start=(kt == 0), stop=(kt == NKT - 1))
                for half in range(2):
                    ro = 64 * half
                    cu = cupool.tile([65, 512], f32, tag="cu")
                    nc.vector.tensor_copy(out=cu, in_=ctx_ab[half])
                    nc.vector.reciprocal(out=cu[64:65, :], in_=cu[64:65, :])
                    zd = zdpool.tile([1, 512], f32, tag="zd", name="zd")
                    nc.sync.dma_start(out=zd, in_=cu[64:65, :])
                    zrow = zd[0, :]
                    rb_src = bass.AP(
                        tensor=zrow.tensor, offset=zrow.offset,
                        ap=[[0, 64]] + list(zrow.ap))
                    rb = rbpool.tile([64, 512], f32, tag="rb")
                    nc.sync.dma_start(out=rb, in_=rb_src)
                    nc.vector.tensor_mul(
                        ctxt[jt][ro:ro + 64, :], cu[0:64, :], rb)
            for s in range(4):
                for e in range(2):
                    es = slice(512 * e, 512 * (e + 1))
                    ps = wop.tile([128, 512], f32, tag="wo")
                    for jt in range(NJ):
                        nc.tensor.matmul(
                            out=ps,
                            lhsT=ctxt[jt][:, 128 * s:128 * (s + 1)],
                            rhs=wo_sb[jt][:, es],
                            start=(jt == 0), stop=(jt == NJ - 1))
                    ob = opool.tile([128, 512], f32, tag="ob")
                    nc.vector.tensor_copy(out=ob, in_=ps)
                    r0 = 512 * t + 128 * s
                    nc.sync.dma_start(out=out_d[r0:r0 + 128, es], in_=ob)

        with (
            tc.tile_pool(name="w3", bufs=1) as w3pool,
            tc.tile_pool(name="xin", bufs=1) as xpool,
        ):
            setup_w3(w3pool)
            for t in range(NT):
                emit_proj(t, xpool)
        mpool = ep(tc.tile_pool(name="msk", bufs=1))
        for t in range(NT):
            emit_attn(t, mpool)

    nc.finalize()
    return nc


def _get_nc(causal):
    if causal not in _cache:
        _cache[causal] = _build_causal() if causal else _build_generic()
    return _cache[causal]


last_result = None


def _is_causal(attn_mask):
    tri = np.tril(np.ones((L, L), bool))
    expect = np.where(tri, np.float32(0.0), np.float32(-1e9))
    return np.array_equal(attn_mask, expect)


def _causal_in_maps(inp):
    from ml_dtypes import bfloat16

    scale = 1.0 / np.sqrt(np.float32(DH))
    wq_s = (inp["Wq"].astype(np.float32) * scale).astype(bfloat16)
    bq_s = (inp["bq"].astype(np.float32) * scale)
    wk_s = inp["Wk"].astype(np.float32).astype(bfloat16)
    wv_s = inp["Wv"].astype(np.float32).astype(bfloat16)
    wo_s = inp["Wo"].astype(np.float32).astype(bfloat16)
    padd = inp["padd_mask"].astype(np.float32)

    kk = np.arange(128)[:, None]
    qq = np.arange(128)[None, :]
    dtri = np.where(qq >= kk, np.float32(0.0),
                    np.float32(-1e9)).astype(np.float32)
    ident = np.eye(128, dtype=np.float32).astype(bfloat16)

    in_maps = []
    for b in range(B):
        xq = np.ascontiguousarray(
            inp["encodings_for_q"][b].astype(np.float32).T).astype(bfloat16)
        xk = np.ascontiguousarray(
            inp["encodings_for_k"][b].astype(np.float32).T).astype(bfloat16)
        xv = np.ascontiguousarray(
            inp["encodings_for_v"][b].astype(np.float32).T).astype(bfloat16)
        pdk = np.ascontiguousarray(padd[b].reshape(NKT, 128).T)
        for g in range(G):
            gs = slice(DG * g, DG * (g + 1))
            in_maps.append({
                "xqt": xq, "xkt": xk, "xvt": xv,
                "wq": np.ascontiguousarray(wq_s[:, gs]),
                "wk": np.ascontiguousarray(wk_s[:, gs]),
                "wv": np.ascontiguousarray(wv_s[:, gs]),
                "bq": np.ascontiguousarray(bq_s[gs].reshape(NJ, 128).T),
                "bk": np.ascontiguousarray(
                    inp["bk"].astype(np.float32)[gs].reshape(NJ, 128).T),
                "bv": np.ascontiguousarray(inp["bv"].astype(np.float32)[gs]),
                "wo": np.ascontiguousarray(wo_s[gs, :]),
                "dtri": dtri,
                "paddk": pdk,
                "ident": ident,
            })
    return in_maps


def _generic_in_maps(inp):
    scale = 1.0 / np.sqrt(np.float32(DH))
    wq_s = (inp["Wq"] * scale).astype(np.float32)
    bq_s = (inp["bq"] * scale).astype(np.float32)
    padd = inp["padd_mask"].astype(np.float32)
    maskT = np.ascontiguousarray(inp["attn_mask"].astype(np.float32).T)

    in_maps = []
    for b in range(B):
        xq = np.ascontiguousarray(inp["encodings_for_q"][b].astype(np.float32).T)
        xk = np.ascontiguousarray(inp["encodings_for_k"][b].astype(np.float32).T)
        xv = np.ascontiguousarray(inp["encodings_for_v"][b].astype(np.float32).T)
        mt = (maskT + padd[b][:, None]).astype(np.float32)
        for g in range(G):
            gs = slice(DG * g, DG * (g + 1))
            in_maps.append({
                "xqt": xq, "xkt": xk, "xvt": xv,
                "wq": np.ascontiguousarray(wq_s[:, gs]),
                "wk": np.ascontiguousarray(inp["Wk"].astype(np.float32)[:, gs]),
                "wv": np.ascontiguousarray(inp["Wv"].astype(np.float32)[:, gs]),
                "bq": np.ascontiguousarray(bq_s[gs].reshape(NJ, 128).T),
                "bk": np.ascontiguousarray(
                    inp["bk"].astype(np.float32)[gs].reshape(NJ, 128).T),
                "bv": np.ascontiguousarray(inp["bv"].astype(np.float32)[gs]),
                "wo": np.ascontiguousarray(inp["Wo"].astype(np.float32)[gs, :]),
                "maskt": mt,
            })
    return in_maps


def kernel(**inputs):
    global last_result
    import os
    from concourse.bass_utils import run_bass_kernel_spmd

    inp = {k: np.asarray(v) for k, v in inputs.items()}
    causal = _is_causal(inp["attn_mask"].astype(np.float32))
    trace = bool(os.environ.get("KBENCH_TRACE"))

    # causal fast path, one retry for transient failures, then the
    # generic-mask fallback; non-finite output counts as failure
    variants = [True, True, False] if causal else [False, False]
    last_exc = None
    for v in variants:
        try:
            nc = _get_nc(v)
            in_maps = (_causal_in_maps if v else _generic_in_maps)(inp)
            res = run_bass_kernel_spmd(
                nc, in_maps, list(range(NCORES)), trace=trace)
            out = np.empty((B, L, D), np.float32)
            for b in range(B):
                out[b] = (res.results[2 * b]["out"]
                          + res.results[2 * b + 1]["out"])
            if not np.isfinite(out).all():
                raise RuntimeError("non-finite output")
            last_result = res
            return out
        except Exception as e:
            last_exc = e
    raise last_exc


# revision 59
# speedup vs baseline: 1.0145x; 1.0027x over previous
"""Multi-head attention (B=4, L=2048, D=1024, H=16) on 8 trn2 NeuronCores.

Sharding: core c = 2*b + g handles batch b and head-group g (8 heads = 512 dims).
Each core computes Q/K/V projections for its group, attention for its 8 heads,
and a partial output projection ctx_g @ Wo[g*512:(g+1)*512, :].  The host sums
the two group partials per batch.

Causal fast path (bf16 PE inputs; ~220.9us tile-cost-model span per core):
  QT, KT  : (512, 2048) feature-major bf16 (4 tiles of (128, L), 2 heads/tile)
  V       : 16 token tiles (128, 8*65) bf16; col 64 of each head = ones column
            that accumulates the softmax denominator Z during the ctx matmul
  scores  : per (head-pair, key-tile) a paired PSUM tile (128, 2, 512) holding
            both heads' score blocks; diagonal key-tiles are column-trimmed to
            the causally-live range and get a (128,2,128) triangular mask add
            on DVE; ONE exp per pair on ACT (bias = padd mask per key)
  ctx     : token-major accumulation — out (128 tok, 65) per 128-query chunk
            (moving dim = 65 features, so ctx matmul cost is ~0.5x of the
            feature-major form); Z lands as a per-partition column, so the
            softmax division is a plain DVE tensor_scalar (no broadcast);
            a tiny PE transpose (identity matmul) restores feature-major
            ctxt for the output projection
  out     : ctxt.T @ Wo chunks into PSUM, DVE/ACT copy, DMA to DRAM f32;
            outproj(t) emission is deferred two blocks and proj(t+1) pieces
            are interleaved between head-pair blocks, so both slot into the
            ACT-bound exp stretches as greedy PE filler; for the final
            block a 2-pass split (jt0-2 into an SBUF partial early, jt3 +
            identity-matmul fold-in late) shortens the tail
  softmax : no max-subtraction (scores are O(3); masked entries hit exp(-1e9)=0)
  x DMAs ride the gpsimd queue, weights/outputs the sync queue, so trigger
  latencies overlap; startup interleaves weight and x chunks per use order.

Non-causal fallback: generic-mask fp32r variant (mask supplied as data).
"""

import sys

if "/opt/trn_rl_repo" not in sys.path:
    sys.path.insert(0, "/opt/trn_rl_repo")

import numpy as np

B, L, D, H = 4, 2048, 1024, 16
G = 2                # head-groups == cores per batch
DG = D // G          # 512 dims per group
HG = H // G          # 8 heads per group
DH = D // H          # 64
NCORES = B * G
NT = L // 512        # query 512-blocks
NKT = L // 128       # key 128-tiles
ND = D // 128        # contraction chunks over input dim
NJ = DG // 128       # dcol tiles per group (2 heads each)

MM_DTYPE = "bfloat16"

_cache = {}


def _build_causal():
    import concourse.bass as bass
    import concourse.tile as tile
    from concourse import bacc, mybir

    f32 = mybir.dt.float32
    f32r = mybir.dt.float32r
    bf16 = mybir.dt.bfloat16
    AF = mybir.ActivationFunctionType

    nc = bacc.Bacc("TRN2")

    xq_d = nc.dram_tensor("xqt", [D, L], bf16, kind="ExternalInput")
    xk_d = nc.dram_tensor("xkt", [D, L], bf16, kind="ExternalInput")
    xv_d = nc.dram_tensor("xvt", [D, L], bf16, kind="ExternalInput")
    wq_d = nc.dram_tensor("wq", [D, DG], bf16, kind="ExternalInput")
    wk_d = nc.dram_tensor("wk", [D, DG], bf16, kind="ExternalInput")
    wv_d = nc.dram_tensor("wv", [D, DG], bf16, kind="ExternalInput")
    bq_d = nc.dram_tensor("bq", [128, NJ], f32, kind="ExternalInput")
    bk_d = nc.dram_tensor("bk", [128, NJ], f32, kind="ExternalInput")
    bv_d = nc.dram_tensor("bv", [DG], f32, kind="ExternalInput")
    wo_d = nc.dram_tensor("wo", [DG, D], bf16, kind="ExternalInput")
    dtri_d = nc.dram_tensor("dtri", [128, 128], f32, kind="ExternalInput")
    pdk_d = nc.dram_tensor("paddk", [128, NKT], f32, kind="ExternalInput")
    id_d = nc.dram_tensor("ident", [128, 128], bf16, kind="ExternalInput")
    out_d = nc.dram_tensor("out", [L, D], f32, kind="ExternalOutput")

    with tile.TileContext(nc) as tc, (
        __import__("contextlib").ExitStack()) as ctx:
        ep = ctx.enter_context
        wpool = ep(tc.tile_pool(name="persist", bufs=1))
        qkpool = ep(tc.tile_pool(name="qk", bufs=1))
        vpool = ep(tc.tile_pool(name="vaug", bufs=1))
        xpool = ep(tc.tile_pool(name="xin", bufs=1))
        scp = ep(tc.tile_pool(name="scps", bufs=2, space="PSUM"))
        cpp = ep(tc.tile_pool(name="ctxps", bufs=2, space="PSUM"))
        gpp = ep(tc.tile_pool(name="gps", bufs=2, space="PSUM"))
        pepool = ep(tc.tile_pool(name="pexp", bufs=3))
        ctpool = ep(tc.tile_pool(name="ctxt", bufs=1))
        rzpool = ep(tc.tile_pool(name="rz", bufs=2))
        opool = ep(tc.tile_pool(name="outsb", bufs=3))

        # ---- persistent weights/biases; DMA order == transfer order, so
        # issue exactly what the first projections need first.
        wq_bg = wpool.tile([128, ND, DG], bf16, tag="wqb", name="wq_bg")
        wk_bg = wpool.tile([128, ND, DG], bf16, tag="wkb", name="wk_bg")
        wv_bg = wpool.tile([128, ND, DG], bf16, tag="wvb", name="wv_bg")
        bq_sb = wpool.tile([128, NJ], f32, tag="bq")
        bk_sb = wpool.tile([128, NJ], f32, tag="bk")
        bv_sb = wpool.tile([128, DG], f32, tag="bv")
        xt = {}
        for name in ("q", "k", "v"):
            xt[name] = [
                xpool.tile([128, ND, 512], bf16, tag=f"x{name}", name=f"x{name}",
                           bufs=3)
                for _ in range(NT)]

        def dma_w_half(wd, wb, hx):
            nc.sync.dma_start(
                out=wb[:, 4 * hx:4 * (hx + 1), :],
                in_=wd[512 * hx:512 * (hx + 1), :].rearrange(
                    "(i p) n -> p i n", p=128))

        def dma_x_chunk(name, xd, t, i, n):
            ts = slice(512 * t, 512 * (t + 1))
            nc.gpsimd.dma_start(
                out=xt[name][t][:, i:i + n, :],
                in_=xd[128 * i:128 * (i + n), ts].rearrange(
                    "(i p) n -> p i n", p=128))

        def dma_x(name, xd, t):
            nc.gpsimd.dma_start(
                out=xt[name][t],
                in_=xd[:, 512 * t:512 * (t + 1)].rearrange(
                    "(i p) n -> p i n", p=128))

        # startup: interleave weight halves with x chunks so the first
        # projection matmuls start as early as possible
        def dma_w_q(wd, wb, i):
            nc.sync.dma_start(
                out=wb[:, i:i + 2, :],
                in_=wd[128 * i:128 * (i + 2), :].rearrange(
                    "(i p) n -> p i n", p=128))

        def dma_w_1(wd, wb, i):
            nc.sync.dma_start(
                out=wb[:, i:i + 1, :],
                in_=wd[128 * i:128 * (i + 1), :].rearrange(
                    "(i p) n -> p i n", p=128))

        dma_w_1(wq_d, wq_bg, 0)
        dma_x_chunk("q", xq_d, 0, 0, 1)
        dma_w_1(wq_d, wq_bg, 1)
        dma_x_chunk("q", xq_d, 0, 1, 1)
        dma_w_q(wq_d, wq_bg, 2)
        dma_x_chunk("q", xq_d, 0, 2, 2)
        dma_w_half(wq_d, wq_bg, 1)
        dma_x_chunk("q", xq_d, 0, 4, 2)
        dma_x_chunk("q", xq_d, 0, 6, 2)
        nc.sync.dma_start(out=bq_sb, in_=bq_d[:, :])
        dma_w_half(wk_d, wk_bg, 0)
        dma_x_chunk("k", xk_d, 0, 0, 2)
        dma_x_chunk("k", xk_d, 0, 2, 2)
        dma_w_half(wk_d, wk_bg, 1)
        dma_x_chunk("k", xk_d, 0, 4, 2)
        dma_x_chunk("k", xk_d, 0, 6, 2)
        nc.sync.dma_start(out=bk_sb, in_=bk_d[:, :])

        dtri = wpool.tile([128, 128], f32, tag="dtri")
        nc.sync.dma_start(out=dtri, in_=dtri_d[:, :])
        pdk_sb = wpool.tile([128, NKT], f32, tag="pdk")
        nc.sync.dma_start(out=pdk_sb, in_=pdk_d[:, :])

        dma_w_half(wv_d, wv_bg, 0)
        dma_x_chunk("v", xv_d, 0, 0, 4)
        dma_w_half(wv_d, wv_bg, 1)
        bv_ap = bv_d[:]
        bv_bcast = bass.AP(
            tensor=bv_ap.tensor, offset=bv_ap.offset,
            ap=[[0, 128]] + list(bv_ap.ap))
        nc.sync.dma_start(out=bv_sb, in_=bv_bcast)
        dma_x_chunk("v", xv_d, 0, 4, 4)

        ident = wpool.tile([128, 128], bf16, tag="ident")
        nc.sync.dma_start(out=ident, in_=id_d[:, :])

        wq_sb = [wq_bg[:, i, :] for i in range(ND)]
        wk_sb = [wk_bg[:, i, :] for i in range(ND)]
        wv_sb = [wv_bg[:, i, :] for i in range(ND)]

        # triangular mask broadcast over the head-pair dim: [128, 2, 128]
        dt_ap = dtri[:, :]
        dtri2 = bass.AP(
            tensor=dt_ap.tensor, offset=dt_ap.offset,
            ap=[dt_ap.ap[0], [0, 2], dt_ap.ap[1]])

        qt_sb = [qkpool.tile([128, L], bf16, tag=f"qt{j}", name="qt_sb")
                 for j in range(NJ)]
        kt_sb = [qkpool.tile([128, L], bf16, tag=f"kt{j}", name="kt_sb")
                 for j in range(NJ)]
        vaug = [vpool.tile([128, HG * 65], bf16, tag=f"va{k}", name="vaug")
                for k in range(NKT)]
        for kt in range(NKT):
            v3 = vaug[kt].rearrange("p (h d) -> p h d", h=HG)
            nc.vector.memset(v3[:, :, 64:65], 1.0)


        wo_bg = wpool.tile([128, NJ, D], bf16, tag="wob", name="wo_bg")
        wo_sb = [wo_bg[:, j, :] for j in range(NJ)]

        def emit_proj(t, only=None):
            ts = slice(512 * t, 512 * (t + 1))
            for name, w_sb, b_sb, dest in (
                ("q", wq_sb, bq_sb, qt_sb),
                ("k", wk_sb, bk_sb, kt_sb),
                ("v", wv_sb, None, None),
            ):
                if only is not None and name != only:
                    continue
                xts = xt[name][t]
                if dest is not None:  # Q/K: feature-major output
                    for j in range(NJ):
                        ps = gpp.tile([128, 512], f32, tag="gp")
                        for i in range(ND):
                            nc.tensor.matmul(
                                out=ps,
                                lhsT=w_sb[i][:, 128 * j:128 * (j + 1)],
                                rhs=xts[:, i, :],
                                start=(i == 0), stop=(i == ND - 1))
                        nc.vector.tensor_scalar_add(
                            out=dest[j][:, ts], in0=ps,
                            scalar1=b_sb[:, j:j + 1])
                else:  # V: token-major output, bv add fused in copy-out
                    for s in range(4):
                        ps = gpp.tile([128, 512], f32, tag="gp")
                        for i in range(ND):
                            nc.tensor.matmul(
                                out=ps,
                                lhsT=xts[:, i, 128 * s:128 * (s + 1)],
                                rhs=wv_sb[i],
                                start=(i == 0), stop=(i == ND - 1))
                        kt = 4 * t + s
                        v3 = vaug[kt].rearrange("p (h d) -> p h d", h=HG)
                        nc.vector.tensor_add(
                            v3[:, :, 0:64],
                            ps.rearrange("p (h d) -> p h d", h=HG),
                            bv_sb.rearrange("p (h d) -> p h d", h=HG))

        def emit_attn(t, fillers=()):
            qs0 = 512 * t
            nkt_t = 4 * t + 4
            ctxt = [ctpool.tile([128, 512], bf16, tag=f"ct{j}", name="ctxt",
                                bufs=4) for j in range(NJ)]
            for hp in range(NJ):
                if hp < len(fillers) and fillers[hp] is not None:
                    fillers[hp]()
                # token-major ctx accumulators: bank X holds query chunks
                # (2X, 2X+1); cols h*65+64 accumulate the softmax denom Z
                # (padded to a full 2048B bank so matmuls stay bank-contained)
                cu = [cpp.tile([128, 2, 256], f32, tag="cu", name="cu",
                               bufs=2) for _ in range(2)]
                for kt in range(nkt_t):
                    ks = slice(128 * kt, 128 * (kt + 1))
                    j = kt - 4 * t  # >= 0 on diagonal tiles
                    o = 128 * j if j >= 0 else 0
                    pair = scp.tile([128, 2, 512], f32, tag="sc", name="scores")
                    for half in range(2):
                        ro = 64 * half
                        nc.tensor.matmul(
                            out=pair[:, half, o:512],
                            lhsT=kt_sb[hp][ro:ro + 64, ks],
                            rhs=qt_sb[hp][ro:ro + 64, qs0 + o:qs0 + 512],
                            start=True, stop=True)
                    if j >= 0:
                        nc.vector.tensor_add(
                            pair[:, :, o:o + 128], pair[:, :, o:o + 128], dtri2)
                    pe = pepool.tile([128, 2, 512], bf16, tag="pe", name="pexp",
                                     bufs=6)
                    nc.scalar.activation(
                        out=pe[:, :, o:512], in_=pair[:, :, o:512],
                        func=AF.Exp, bias=pdk_sb[:, kt:kt + 1])
                    c0 = max(j, 0)
                    for c in range(c0, 4):
                        for half in range(2):
                            h = 2 * hp + half
                            # start zeroes the full 2KB PSUM zero-region, so
                            # only the first matmul into each bank sets it
                            nc.tensor.matmul(
                                out=cu[c // 2][:, c % 2, 65 * half:65 * half + 65],
                                lhsT=pe[:, half, 128 * c:128 * (c + 1)],
                                rhs=vaug[kt][:, 65 * h:65 * (h + 1)],
                                start=(kt == 0 and half == 0 and c % 2 == 0),
                                stop=(kt == 4 * t + 2 * (c // 2) + 1
                                      and c % 2 == 1 and half == 1))
                # normalize: Z sits as per-partition columns; recip + scalar
                # multiply, then PE-transpose back to feature-major ctxt
                cub = ctpool.tile([128, 4, 128], bf16, tag="cub", bufs=2)
                last = (t == NT - 1 and hp == NJ - 1)
                for X in range(2):
                    cu4 = cu[X][:, :, 0:130].rearrange(
                        "p c (h f) -> p c h f", f=65)
                    rz = rzpool.tile([128, 2, 2], f32, tag="rz", bufs=4)
                    nc.vector.reciprocal(out=rz, in_=cu4[:, :, :, 64])
                    for cc in range(2):
                        for half in range(2):
                            co = cub[:, 2 * X + cc, 64 * half:64 * half + 64]
                            if last and half == 1:
                                # ACT is idle at the very end; split the
                                # normalize between both engines
                                nc.scalar.mul(
                                    co, cu4[:, cc, half, 0:64],
                                    rz[:, cc, half:half + 1])
                            else:
                                nc.vector.tensor_scalar_mul(
                                    out=co, in0=cu4[:, cc, half, 0:64],
                                    scalar1=rz[:, cc, half:half + 1])
                tr = cpp.tile([128, 1024], bf16, tag="cu", name="tr")
                for c in range(4):
                    nc.tensor.matmul(
                        out=tr[:, 128 * c:128 * (c + 1)],
                        lhsT=cub[:, c, :],
                        rhs=ident,
                        is_transpose=True,
                        start=(c == 0), stop=(c == 3), skip_group_check=True)
                if last:
                    # final head-pair: split the drain across DVE and ACT so
                    # the last output-projection pass starts sooner
                    nc.vector.tensor_copy(
                        out=ctxt[hp][:, 0:256], in_=tr[:, 0:256])
                    nc.scalar.copy(
                        out=ctxt[hp][:, 256:512], in_=tr[:, 256:512])
                else:
                    nc.vector.tensor_copy(out=ctxt[hp], in_=tr[:, 0:512])
            return ctxt

        def emit_outproj(t, ctxt, split=False):
            # split=True: accumulate jt0-2 into an SBUF partial early (fills
            # PE while the last head-pairs' exps drain), only jt3 + add +
            # DMA remain after the final ctxt lands
            parts = {}
            if split:
                for s in range(4):
                    for e in range(2):
                        es = slice(512 * e, 512 * (e + 1))
                        ps = gpp.tile([128, 512], f32, tag="gp")
                        for jt in range(3):
                            nc.tensor.matmul(
                                out=ps,
                                lhsT=ctxt[jt][:, 128 * s:128 * (s + 1)],
                                rhs=wo_sb[jt][:, es],
                                start=(jt == 0), stop=(jt == 2))
                        oa = opool.tile([128, 512], bf16, tag="oa", bufs=8)
                        nc.vector.tensor_copy(out=oa, in_=ps)
                        parts[s, e] = oa
            if split:
                # final pass on sc pair tiles (scores are done): jt3 plus the
                # SBUF partial folded back via identity matmuls, all on PE
                for s in range(4):
                    pr = scp.tile([128, 2, 512], f32, tag="sc", name="opr")
                    for e in range(2):
                        es = slice(512 * e, 512 * (e + 1))
                        # fold the early SBUF partial in FIRST (ready long
                        # before ctxt[3]) so only one matmul remains after
                        # the final head-pair's normalize lands
                        nc.tensor.matmul(
                            out=pr[:, e, :], lhsT=ident, rhs=parts[s, e],
                            start=True, stop=False)
                        nc.tensor.matmul(
                            out=pr[:, e, :],
                            lhsT=ctxt[3][:, 128 * s:128 * (s + 1)],
                            rhs=wo_sb[3][:, es],
                            start=False, stop=True)
                    r0 = 512 * t + 128 * s
                    for e in range(2):
                        es = slice(512 * e, 512 * (e + 1))
                        ob = opool.tile([128, 512], f32, tag="ob", bufs=6)
                        if e == 0:
                            nc.vector.tensor_copy(out=ob, in_=pr[:, e, :])
                        else:
                            nc.scalar.copy(out=ob, in_=pr[:, e, :])
                        if s == 3:
                            # last pair: halve the final DMAs across queues
                            for h in range(2):
                                q = nc.gpsimd if (e + h) % 2 else nc.sync
                                q.dma_start(
                                    out=out_d[r0:r0 + 128,
                                              512 * e + 256 * h:
                                              512 * e + 256 * (h + 1)],
                                    in_=ob[:, 256 * h:256 * (h + 1)])
                        else:
                            q = nc.gpsimd if e == 1 else nc.sync
                            q.dma_start(out=out_d[r0:r0 + 128, es], in_=ob)
            else:
                for s in range(4):
                    for e in range(2):
                        es = slice(512 * e, 512 * (e + 1))
                        ps = gpp.tile([128, 512], f32, tag="gp")
                        for jt in range(NJ):
                            nc.tensor.matmul(
                                out=ps,
                                lhsT=ctxt[jt][:, 128 * s:128 * (s + 1)],
                                rhs=wo_sb[jt][:, es],
                                start=(jt == 0), stop=(jt == NJ - 1))
                        # DVE-only drains: these outprojs run as filler in
                        # exp-saturated attention windows, so an ACT drain
                        # would delay the exp stream directly; halved to
                        # reduce head-of-line blocking of mask adds on DVE
                        ob = opool.tile([128, 512], f32, tag="ob", bufs=6)
                        nc.vector.tensor_copy(out=ob[:, 0:256], in_=ps[:, 0:256])
                        nc.vector.tensor_copy(
                            out=ob[:, 256:512], in_=ps[:, 256:512])
                        r0 = 512 * t + 128 * s
                        nc.sync.dma_start(out=out_d[r0:r0 + 128, es], in_=ob)

        emit_proj(0)
        for name, xd in (("q", xq_d), ("k", xk_d), ("v", xv_d)):
            dma_x(name, xd, 1)
        nc.sync.dma_start(
            out=wo_bg, in_=wo_d[:, :].rearrange("(j p) n -> p j n", p=128))
        # outproj(t) is deferred two attention blocks: it becomes greedy PE
        # filler for the ACT-bound stretches of the last attention blocks
        pend = []
        for t in range(NT):
            # proj(t+1) pieces are interleaved between head-pair blocks so
            # their priority slots them into the ACT-bound exp stretches
            if t + 1 < NT:
                fillers = [None] + [
                    (lambda n=n: emit_proj(t + 1, only=n))
                    for n in ("q", "k", "v")]
            else:
                fillers = ()
            pend.append((t, emit_attn(t, fillers)))
            if t + 1 < NT and t + 2 < NT:
                for name, xd in (("q", xq_d), ("k", xk_d), ("v", xv_d)):
                    dma_x(name, xd, t + 2)
            if t >= 2:
                emit_outproj(*pend.pop(0))
        while len(pend) > 1:
            emit_outproj(*pend.pop(0))
        emit_outproj(*pend.pop(0), split=True)

    nc.finalize()
    return nc


def _build_generic():
    """Non-causal fallback: generic additive mask as data, fp32r PE."""
    import concourse.bass as bass
    import concourse.tile as tile
    from concourse import bacc, mybir

    f32 = mybir.dt.float32
    AF = mybir.ActivationFunctionType
    mdt = mybir.dt.float32r

    nc = bacc.Bacc("TRN2")

    xqt = nc.dram_tensor("xqt", [D, L], mdt, kind="ExternalInput")
    xkt = nc.dram_tensor("xkt", [D, L], mdt, kind="ExternalInput")
    xvt = nc.dram_tensor("xvt", [D, L], mdt, kind="ExternalInput")
    wq_d = nc.dram_tensor("wq", [D, DG], mdt, kind="ExternalInput")
    wk_d = nc.dram_tensor("wk", [D, DG], mdt, kind="ExternalInput")
    wv_d = nc.dram_tensor("wv", [D, DG], mdt, kind="ExternalInput")
    bq_d = nc.dram_tensor("bq", [128, NJ], f32, kind="ExternalInput")
    bk_d = nc.dram_tensor("bk", [128, NJ], f32, kind="ExternalInput")
    bv_d = nc.dram_tensor("bv", [DG], f32, kind="ExternalInput")
    wo_d = nc.dram_tensor("wo", [DG, D], mdt, kind="ExternalInput")
    msk_d = nc.dram_tensor("maskt", [L, L], f32, kind="ExternalInput")
    out_d = nc.dram_tensor("out", [L, D], f32, kind="ExternalOutput")

    with tile.TileContext(nc) as tc, (
        __import__("contextlib").ExitStack()) as ctx:
        ep = ctx.enter_context
        wpool = ep(tc.tile_pool(name="persist", bufs=1))
        qkpool = ep(tc.tile_pool(name="qk", bufs=1))
        vpool = ep(tc.tile_pool(name="vaug", bufs=1))
        zdpool = ep(tc.tile_pool(name="zdram", bufs=4, space="DRAM"))
        mmp = ep(tc.tile_pool(name="mm", bufs=4, space="PSUM"))
        ctxp = ep(tc.tile_pool(name="ctxps", bufs=2, space="PSUM"))
        wop = ep(tc.tile_pool(name="wops", bufs=2, space="PSUM"))
        ppool = ep(tc.tile_pool(name="pexp", bufs=3))
        ctpool = ep(tc.tile_pool(name="ctxt", bufs=4))
        cupool = ep(tc.tile_pool(name="ctxu", bufs=2))
        rbpool = ep(tc.tile_pool(name="rbc", bufs=1))
        opool = ep(tc.tile_pool(name="outsb", bufs=2))

        wo_bg = wpool.tile([128, NJ, D], mdt, tag="wob", name="wo_bg")
        nc.sync.dma_start(
            out=wo_bg, in_=wo_d[:, :].rearrange("(j p) n -> p j n", p=128))
        wo_sb = [wo_bg[:, j, :] for j in range(NJ)]
        qt_sb = [qkpool.tile([128, L], mdt, tag=f"qt{j}", name="qt_sb") for j in range(NJ)]
        kt_sb = [qkpool.tile([128, L], mdt, tag=f"kt{j}", name="kt_sb") for j in range(NJ)]
        vaug = [vpool.tile([128, HG * 65], mdt, tag=f"va{k}", name="vaug") for k in range(NKT)]
        ones8 = wpool.tile([128, HG, 1], f32, tag="ones8")
        nc.vector.memset(ones8, 1.0)
        for kt in range(NKT):
            v3 = vaug[kt].rearrange("p (h d) -> p h d", h=HG)
            nc.scalar.copy(out=v3[:, :, 64:65], in_=ones8)

        wq_sb = wk_sb = wv_sb = bq_sb = bk_sb = bv_sb = None

        def setup_w3(w3pool):
            nonlocal wq_sb, wk_sb, wv_sb, bq_sb, bk_sb, bv_sb
            wq_bg = w3pool.tile([128, ND, DG], mdt, tag="wqb", name="wq_bg")
            wk_bg = w3pool.tile([128, ND, DG], mdt, tag="wkb", name="wk_bg")
            wv_bg = w3pool.tile([128, ND, DG], mdt, tag="wvb", name="wv_bg")
            for wd, wb in ((wq_d, wq_bg), (wk_d, wk_bg), (wv_d, wv_bg)):
                nc.sync.dma_start(
                    out=wb, in_=wd[:, :].rearrange("(i p) n -> p i n", p=128))
            wq_sb = [wq_bg[:, i, :] for i in range(ND)]
            wk_sb = [wk_bg[:, i, :] for i in range(ND)]
            wv_sb = [wv_bg[:, i, :] for i in range(ND)]
            bq_sb = w3pool.tile([128, NJ], f32, tag="bq")
            bk_sb = w3pool.tile([128, NJ], f32, tag="bk")
            nc.sync.dma_start(out=bq_sb, in_=bq_d[:, :])
            nc.sync.dma_start(out=bk_sb, in_=bk_d[:, :])
            bv_sb = w3pool.tile([128, DG], f32, tag="bv")
            bv_ap = bv_d[:]
            bv_bcast = bass.AP(
                tensor=bv_ap.tensor, offset=bv_ap.offset,
                ap=[[0, 128]] + list(bv_ap.ap))
            nc.sync.dma_start(out=bv_sb, in_=bv_bcast)

        def emit_proj(t, xpool):
            ts = slice(512 * t, 512 * (t + 1))
            for xd, w_sb, b_sb, dest in (
                (xqt, wq_sb, bq_sb, qt_sb),
                (xkt, wk_sb, bk_sb, kt_sb),
                (xvt, wv_sb, None, None),
            ):
                xts = []
                for hx in range(4):
                    xt_bg = xpool.tile([128, ND // 4, 512], mdt, tag="xt",
                                       name="xt_bg", bufs=3)
                    rs = slice(256 * hx, 256 * (hx + 1))
                    nc.sync.dma_start(
                        out=xt_bg,
                        in_=xd[rs, ts].rearrange("(i p) n -> p i n", p=128))
                    xts.extend(xt_bg[:, i, :] for i in range(ND // 4))
                if dest is not None:  # Q/K: feature-major output
                    for j in range(NJ):
                        ps = mmp.tile([128, 512], f32, tag="mm")
                        for i in range(ND):
                            nc.tensor.matmul(
                                out=ps,
                                lhsT=w_sb[i][:, 128 * j:128 * (j + 1)],
                                rhs=xts[i],
                                start=(i == 0), stop=(i == ND - 1))
                        nc.scalar.activation(
                            out=dest[j][:, ts], in_=ps, func=AF.Identity,
                            bias=b_sb[:, j:j + 1])
                else:  # V: token-major output, bv add fused in copy-out
                    for s in range(4):
                        ps = mmp.tile([128, 512], f32, tag="mm")
                        for i in range(ND):
                            nc.tensor.matmul(
                                out=ps,
                                lhsT=xts[i][:, 128 * s:128 * (s + 1)],
                                rhs=wv_sb[i],
                                start=(i == 0), stop=(i == ND - 1))
                        kt = 4 * t + s
                        v3 = vaug[kt].rearrange("p (h d) -> p h d", h=HG)
                        nc.vector.tensor_add(
                            v3[:, :, 0:64],
                            ps.rearrange("p (h d) -> p h d", h=HG),
                            bv_sb.rearrange("p (h d) -> p h d", h=HG))

        def emit_attn(t, mpool):
            qs = slice(512 * t, 512 * (t + 1))
            msk = []
            for hkt in range(4):
                msk_bg = mpool.tile([128, NKT // 4, 512], f32, tag="msk",
                                    name="msk_bg", bufs=6)
                rs = slice(512 * hkt, 512 * (hkt + 1))
                nc.sync.dma_start(
                    out=msk_bg,
                    in_=msk_d[rs, qs].rearrange("(k p) n -> p k n", p=128))
                msk.extend(msk_bg[:, kt, :] for kt in range(NKT // 4))
            ctxt = [ctpool.tile([128, 512], mdt, tag="ct", name="ctxt") for _ in range(NJ)]
            for hp in range(NJ):
                jt = hp
                ctx_ab = [ctxp.tile([65, 512], f32, tag="ctx", name="ctx_ab") for _ in range(2)]
                for kt in range(NKT):
                    ks = slice(128 * kt, 128 * (kt + 1))
                    pexp = []
                    for half in range(2):
                        ro = 64 * half
                        ps = mmp.tile([128, 512], f32, tag="mm")
                        nc.tensor.matmul(
                            out=ps,
                            lhsT=kt_sb[jt][ro:ro + 64, ks],
                            rhs=qt_sb[jt][ro:ro + 64, qs],
                            start=True, stop=True)
                        nc.vector.tensor_add(ps, ps, msk[kt])
                        pe = ppool.tile([128, 512], mdt, tag="pexp")
                        nc.scalar.activation(out=pe, in_=ps, func=AF.Exp, bias=0.0)
                        pexp.append(pe)
                    for half in range(2):
                        h = 2 * hp + half
                        nc.tensor.matmul(
                            out=ctx_ab[half],
                            lhsT=vaug[kt][:, 65 * h:65 * (h + 1)],
                            rhs=pexp[half],
                            start=(kt == 0), stop=(kt == NKT - 1))
                for half in range(2):
                    ro = 64 * half
                    cu = cupool.tile([65, 512], f32, tag="cu")
                    nc.vector.tensor_copy(out=cu, in_=ctx_ab[half])
                    nc.vector.reciprocal(out=cu[64:65, :], in_=cu[64:65, :])
                    zd = zdpool.tile([1, 512], f32, tag="zd", name="zd")
                    nc.sync.dma_start(out=zd, in_=cu[64:65, :])
                    zrow = zd[0, :]
                    rb_src = bass.AP(
                        tensor=zrow.tensor, offset=zrow.offset,
                        ap=[[0, 64]] + list(zrow.ap))
                    rb = rbpool.tile([64, 512], f32, tag="rb")
                    nc.sync.dma_start(out=rb, in_=rb_src)
                    nc.vector.tensor_mul(
                        ctxt[jt][ro:ro + 64, :], cu[0:64, :], rb)
            for s in range(4):
                for e in range(2):
                    es = slice(512 * e, 512 * (e + 1))
                    ps = wop.tile([128, 512], f32, tag="wo")
                    for jt in range(NJ):
                        nc.tensor.matmul(
                            out=ps,
                            lhsT=ctxt[jt][:, 128 * s:128 * (s + 1)],
                            rhs=wo_sb[jt][:, es],
                            start=(jt == 0), stop=(jt == NJ - 1))
                    ob = opool.tile([128, 512], f32, tag="ob")
                    nc.vector.tensor_copy(out=ob, in_=ps)
                    r0 = 512 * t + 128 * s
                    nc.sync.dma_start(out=out_d[r0:r0 + 128, es], in_=ob)

        with (
            tc.tile_pool(name="w3", bufs=1) as w3pool,
            tc.tile_pool(name="xin", bufs=1) as xpool,
        ):
            setup_w3(w3pool)
            for t in range(NT):
                emit_proj(t, xpool)
        mpool = ep(tc.tile_pool(name="msk", bufs=1))
        for t in range(NT):
            emit_attn(t, mpool)

    nc.finalize()
    return nc


def _get_nc(causal):
    if causal not in _cache:
        _cache[causal] = _build_causal() if causal else _build_generic()
    return _cache[causal]


last_result = None


def _is_causal(attn_mask):
    tri = np.tril(np.ones((L, L), bool))
    expect = np.where(tri, np.float32(0.0), np.float32(-1e9))
    return np.array_equal(attn_mask, expect)


def _causal_in_maps(inp):
    from ml_dtypes import bfloat16

    scale = 1.0 / np.sqrt(np.float32(DH))
    wq_s = (inp["Wq"].astype(np.float32) * scale).astype(bfloat16)
    bq_s = (inp["bq"].astype(np.float32) * scale)
    wk_s = inp["Wk"].astype(np.float32).astype(bfloat16)
    wv_s = inp["Wv"].astype(np.float32).astype(bfloat16)
    wo_s = inp["Wo"].astype(np.float32).astype(bfloat16)
    padd = inp["padd_mask"].astype(np.float32)

    kk = np.arange(128)[:, None]
    qq = np.arange(128)[None, :]
    dtri = np.where(qq >= kk, np.float32(0.0),
                    np.float32(-1e9)).astype(np.float32)
    ident = np.eye(128, dtype=np.float32).astype(bfloat16)

    in_maps = []
    for b in range(B):
        xq = np.ascontiguousarray(
            inp["encodings_for_q"][b].astype(np.float32).T).astype(bfloat16)
        xk = np.ascontiguousarray(
            inp["encodings_for_k"][b].astype(np.float32).T).astype(bfloat16)
        xv = np.ascontiguousarray(
            inp["encodings_for_v"][b].astype(np.float32).T).astype(bfloat16)
        pdk = np.ascontiguousarray(padd[b].reshape(NKT, 128).T)
        for g in range(G):
            gs = slice(DG * g, DG * (g + 1))
            in_maps.append({
                "xqt": xq, "xkt": xk, "xvt": xv,
                "wq": np.ascontiguousarray(wq_s[:, gs]),
                "wk": np.ascontiguousarray(wk_s[:, gs]),
                "wv": np.ascontiguousarray(wv_s[:, gs]),
                "bq": np.ascontiguousarray(bq_s[gs].reshape(NJ, 128).T),
                "bk": np.ascontiguousarray(
                    inp["bk"].astype(np.float32)[gs].reshape(NJ, 128).T),
                "bv": np.ascontiguousarray(inp["bv"].astype(np.float32)[gs]),
                "wo": np.ascontiguousarray(wo_s[gs, :]),
                "dtri": dtri,
                "paddk": pdk,
                "ident": ident,
            })
    return in_maps


def _generic_in_maps(inp):
    scale = 1.0 / np.sqrt(np.float32(DH))
    wq_s = (inp["Wq"] * scale).astype(np.float32)
    bq_s = (inp["bq"] * scale).astype(np.float32)
    padd = inp["padd_mask"].astype(np.float32)
    maskT = np.ascontiguousarray(inp["attn_mask"].astype(np.float32).T)

    in_maps = []
    for b in range(B):
        xq = np.ascontiguousarray(inp["encodings_for_q"][b].astype(np.float32).T)
        xk = np.ascontiguousarray(inp["encodings_for_k"][b].astype(np.float32).T)
        xv = np.ascontiguousarray(inp["encodings_for_v"][b].astype(np.float32).T)
        mt = (maskT + padd[b][:, None]).astype(np.float32)
        for g in range(G):
            gs = slice(DG * g, DG * (g + 1))
            in_maps.append({
                "xqt": xq, "xkt": xk, "xvt": xv,
                "wq": np.ascontiguousarray(wq_s[:, gs]),
                "wk": np.ascontiguousarray(inp["Wk"].astype(np.float32)[:, gs]),
                "wv": np.ascontiguousarray(inp["Wv"].astype(np.float32)[:, gs]),
                "bq": np.ascontiguousarray(bq_s[gs].reshape(NJ, 128).T),
                "bk": np.ascontiguousarray(
                    inp["bk"].astype(np.float32)[gs].reshape(NJ, 128).T),
                "bv": np.ascontiguousarray(inp["bv"].astype(np.float32)[gs]),
                "wo": np.ascontiguousarray(inp["Wo"].astype(np.float32)[gs, :]),
                "maskt": mt,
            })
    return in_maps


def kernel(**inputs):
    global last_result
    import os
    from concourse.bass_utils import run_bass_kernel_spmd

    inp = {k: np.asarray(v) for k, v in inputs.items()}
    causal = _is_causal(inp["attn_mask"].astype(np.float32))
    trace = bool(os.environ.get("KBENCH_TRACE"))

    # causal fast path, one retry for transient failures, then the
    # generic-mask fallback; non-finite output counts as failure
    variants = [True, True, False] if causal else [False, False]
    last_exc = None
    for v in variants:
        try:
            nc = _get_nc(v)
            in_maps = (_causal_in_maps if v else _generic_in_maps)(inp)
            res = run_bass_kernel_spmd(
                nc, in_maps, list(range(NCORES)), trace=trace)
            out = np.empty((B, L, D), np.float32)
            for b in range(B):
                out[b] = (res.results[2 * b]["out"]
                          + res.results[2 * b + 1]["out"])
            if not np.isfinite(out).all():
                raise RuntimeError("non-finite output")
            last_result = res
            return out
        except Exception as e:
            last_exc = e
    raise last_exc
